# revision 1
# baseline (speedup 1.0000x reference)
"""Exaone4 attention kernel for 8 Trainium2 NeuronCores.

Sharding: tensor-parallel over heads (TP=8). Core i owns query heads
4i..4i+3 and kv head i (one GQA group), processes both batch elements,
and computes a row-parallel partial of the output projection; the host
sums the 8 partials.

Shapes (hardcoded): B=2, S=2048, H=4096, NH=32, NKV=8, D=128,
WINDOW=1024, eps=1e-5, theta=10000.
"""

import os
import sys

for _p in ("/opt/trn_rl_repo",):
    if _p not in sys.path and os.path.isdir(_p):
        sys.path.insert(0, _p)

import numpy as np

B, S, H = 2, 2048, 4096
NH, NKV, D = 32, 8, 128
WINDOW = 1024
EPS = 1e-5
THETA = 10000.0

NCORES = 8
HPC = NH // NCORES          # query heads per core = 4
QW = HPC * D                # q-proj cols per core = 512
CH = 512                    # sequence chunk
NSC = S // CH               # 4 chunks
HC = H // 128               # 32 contraction chunks
NEG = -1.0e30

_CACHE = {}


def _build():
    import concourse.bass as bass
    import concourse.tile as tile
    from concourse import mybir, bacc

    F32 = mybir.dt.float32
    F32R = mybir.dt.float32r
    EXP = mybir.ActivationFunctionType.Exp
    RSQRT = mybir.ActivationFunctionType.Abs_reciprocal_sqrt

    nc = bacc.Bacc("TRN2", target_bir_lowering=False, debug=False)

    hsT = nc.dram_tensor("hsT", [B, H, S], F32R, kind="ExternalInput")
    wq_s = nc.dram_tensor("wq_s", [H, QW], F32R, kind="ExternalInput")
    wk_s = nc.dram_tensor("wk_s", [H, D], F32R, kind="ExternalInput")
    wv_s = nc.dram_tensor("wv_s", [H, D], F32R, kind="ExternalInput")
    wo_s = nc.dram_tensor("wo_s", [QW, H], F32R, kind="ExternalInput")
    cosT = nc.dram_tensor("cosT", [D, S], F32, kind="ExternalInput")
    sinT = nc.dram_tensor("sinT", [D, S], F32, kind="ExternalInput")
    wrow_q = nc.dram_tensor("wrow_q", [1, D], F32R, kind="ExternalInput")
    wrow_k = nc.dram_tensor("wrow_k", [1, D], F32R, kind="ExternalInput")
    ones128 = nc.dram_tensor("ones128", [128, 1], F32R, kind="ExternalInput")
    ones_col = nc.dram_tensor("ones_col", [1, 128], F32R, kind="ExternalInput")
    protT = nc.dram_tensor("protT", [128, 128], F32R, kind="ExternalInput")
    idn = nc.dram_tensor("idn", [128, 128], F32, kind="ExternalInput")
    mask_c = nc.dram_tensor("mask_c", [128, 128], F32, kind="ExternalInput")
    mask_w = nc.dram_tensor("mask_w", [128, 128], F32, kind="ExternalInput")
    out_part = nc.dram_tensor("out_part", [B, S, H], F32, kind="ExternalOutput")

    DEBUG = bool(os.environ.get("BASS_KERNEL_DEBUG"))
    if DEBUG:
        dbg_k = nc.dram_tensor("dbg_k", [128, S], F32, kind="ExternalOutput")
        dbg_v = nc.dram_tensor("dbg_v", [128, S // 128, 128], F32,
                               kind="ExternalOutput")
        dbg_q = nc.dram_tensor("dbg_q", [HPC, 128, S], F32,
                               kind="ExternalOutput")
        dbg_a = nc.dram_tensor("dbg_a", [HPC, 128, S], F32,
                               kind="ExternalOutput")

    with tile.TileContext(nc) as tc, \
         nc.allow_low_precision(reason="deliberate fp32r matmul pipeline"):
        with tc.tile_pool(name="consts", bufs=1) as consts, \
             tc.tile_pool(name="dram", bufs=2, space="DRAM") as dram:
            cos_sb = consts.tile([D, S], F32)
            nc.sync.dma_start(cos_sb, cosT.ap())
            sin_sb = consts.tile([D, S], F32)
            nc.sync.dma_start(sin_sb, sinT.ap())
            wq_row = consts.tile([1, D], F32R)
            nc.sync.dma_start(wq_row, wrow_q.ap())
            wk_row = consts.tile([1, D], F32R)
            nc.sync.dma_start(wk_row, wrow_k.ap())
            on128 = consts.tile([128, 1], F32R)
            nc.sync.dma_start(on128, ones128.ap())
            oncol = consts.tile([1, 128], F32R)
            nc.sync.dma_start(oncol, ones_col.ap())
            prot = consts.tile([128, 128], F32R)
            nc.sync.dma_start(prot, protT.ap())
            iden = consts.tile([128, 128], F32)
            nc.sync.dma_start(iden, idn.ap())
            mc = consts.tile([128, 128], F32)
            nc.sync.dma_start(mc, mask_c.ap())
            mw = consts.tile([128, 128], F32)
            nc.sync.dma_start(mw, mask_w.ap())
            eps_t = consts.tile([1, 1], F32)
            nc.vector.memset(eps_t, EPS)

            scr = [dram.tile([HPC, 128, S], F32R, tag="attn_scr",
                             name=f"attn_scr{b}") for b in range(B)]

            # ---------------- phases A+B: QKV + norm/rope + attention ----
            with tc.tile_pool(name="wqkv", bufs=1) as wp, \
                 tc.tile_pool(name="kv", bufs=1) as kvp, \
                 tc.tile_pool(name="work", bufs=2) as wrk, \
                 tc.tile_pool(name="epi", bufs=1) as epi, \
                 tc.tile_pool(name="hs", bufs=3) as hsp, \
                 tc.tile_pool(name="probs", bufs=3) as prp, \
                 tc.tile_pool(name="qf", bufs=2) as qfp, \
                 tc.tile_pool(name="ps_qkv", bufs=1, space="PSUM") as pq, \
                 tc.tile_pool(name="ps_s", bufs=3, space="PSUM") as pss, \
                 tc.tile_pool(name="ps_o", bufs=1, space="PSUM") as pso, \
                 tc.tile_pool(name="ps_sum", bufs=1, space="PSUM") as psum_p:
                paux = pss

                wq_sb = wp.tile([128, HC, QW], F32R)
                nc.sync.dma_start(
                    wq_sb, wq_s.ap().rearrange("(o p) c -> p o c", p=128))
                wk_sb = wp.tile([128, HC, D], F32R)
                nc.sync.dma_start(
                    wk_sb, wk_s.ap().rearrange("(o p) c -> p o c", p=128))
                wv_sb = wp.tile([128, HC, D], F32R)
                nc.sync.dma_start(
                    wv_sb, wv_s.ap().rearrange("(o p) c -> p o c", p=128))

                for b in range(B):
                    # K in [D, S]; V in [S, D] (by 128-row tiles)
                    k_full = kvp.tile([128, S], F32R, tag="k_full")
                    v_full = kvp.tile([128, S // 128, 128], F32R, tag="v_full")

                    for sc in range(NSC):
                        s0 = CH * sc
                        qfin = []
                        # ---- QKV projection: 2 groups of 3 col-blocks;
                        # each block's psum slice is a full bank.
                        for grp in range(2):
                          with nc.named_scope("qkv"):
                            qkv_ps = pq.tile([128, 3, CH], F32, tag="qkv",
                                             name=f"qkv{grp}")
                            for hc in range(HC):
                                ht = hsp.tile([128, CH], F32R, tag="hst")
                                nc.sync.dma_start(
                                    ht, hsT.ap()[b, 128 * hc:128 * (hc + 1),
                                                 s0:s0 + CH])
                                for bi in range(3):
                                    blk = 3 * grp + bi
                                    if blk < HPC:
                                        lhs = wq_sb[:, hc,
                                                    128 * blk:128 * (blk + 1)]
                                    elif blk == HPC:
                                        lhs = wk_sb[:, hc, :]
                                    else:
                                        lhs = wv_sb[:, hc, :]
                                    nc.tensor.matmul(
                                        qkv_ps[:, bi, :], lhs, ht,
                                        start=(hc == 0), stop=(hc == HC - 1))

                            # ---- epilogue for this group's blocks,
                            # software-pipelined by stage so PE / ACT / DVE
                            # overlap across the 3 blocks.
                            raws, sqs, ssqs, rinvs = {}, {}, {}, {}
                            wtils, qhats, rots = {}, {}, {}
                            bis = list(range(3))
                            for bi in bis:
                                raw = epi.tile([128, CH], F32, tag=f"raw{bi}",
                                               name=f"raw{bi}")
                                nc.vector.tensor_copy(raw, qkv_ps[:, bi, :])
                                raws[bi] = raw
                            norm_bis = [bi for bi in bis
                                        if 3 * grp + bi != HPC + 1]
                            for bi in norm_bis:
                                sq = epi.tile([128, CH], F32R, tag=f"sq{bi}",
                                              name=f"sq{bi}")
                                nc.vector.tensor_mul(sq, raws[bi], raws[bi])
                                sqs[bi] = sq
                            for bi in norm_bis:
                                ssq = paux.tile([1, CH], F32, tag="s",
                                                name=f"ssq{bi}")
                                nc.tensor.matmul(ssq, on128, sqs[bi],
                                                 start=True, stop=True)
                                ssqs[bi] = ssq
                            for bi in norm_bis:
                                rinv = epi.tile([1, CH], F32R, tag=f"rinv{bi}",
                                                name=f"rinv{bi}")
                                nc.scalar.activation(rinv, ssqs[bi], RSQRT,
                                                     bias=eps_t, scale=1.0 / D)
                                rinvs[bi] = rinv
                            for bi in norm_bis:
                                wrow = (wq_row if 3 * grp + bi < HPC
                                        else wk_row)
                                wtil = paux.tile([128, CH], F32, tag="s",
                                                 name=f"wtil{bi}")
                                nc.tensor.matmul(wtil, wrow, rinvs[bi],
                                                 start=True, stop=True)
                                wtils[bi] = wtil
                            for bi in norm_bis:
                                qhat = epi.tile([128, CH], F32R, tag=f"qhat{bi}",
                                                name=f"qhat{bi}")
                                nc.vector.tensor_mul(qhat, wtils[bi],
                                                     raws[bi])
                                qhats[bi] = qhat
                            for bi in norm_bis:
                                rot = paux.tile([128, CH], F32, tag="s",
                                                name=f"rot{bi}")
                                nc.tensor.matmul(rot, prot, qhats[bi],
                                                 start=True, stop=True)
                                rots[bi] = rot
                            for bi in norm_bis:
                                blk = 3 * grp + bi
                                t1 = wrk.tile([128, CH], F32, tag="t1",
                                              name=f"t1_{bi}")
                                nc.vector.tensor_mul(t1, qhats[bi],
                                                     cos_sb[:, s0:s0 + CH])
                                t2 = wrk.tile([128, CH], F32, tag="t2",
                                              name=f"t2_{bi}")
                                nc.vector.tensor_mul(t2, rots[bi],
                                                     sin_sb[:, s0:s0 + CH])
                                if blk < HPC:
                                    qf = qfp.tile([128, CH], F32R,
                                                  tag=f"qfin{blk}")
                                    qfin.append(qf)
                                    nc.vector.tensor_add(qf, t1, t2)
                                else:
                                    nc.vector.tensor_add(
                                        k_full[:, s0:s0 + CH], t1, t2)
                            if grp == 1:
                                # V: transpose [D, S]-chunk to [S, D] tiles
                                vraw = raws[2]
                                for j in range(CH // 128):
                                    tp = paux.tile([128, 128], F32, tag="s",
                                                   name=f"tp{j}")
                                    nc.tensor.transpose(
                                        tp, vraw[:, 128 * j:128 * (j + 1)],
                                        iden)
                                    nc.vector.tensor_copy(
                                        v_full[:, (CH // 128) * sc + j, :],
                                        tp)

                        # ---- attention for query chunk sc (512 queries)
                        kis = list(range(max(0, 4 * sc - 8), 4 * sc + 4))
                        for h in range(HPC):
                          with nc.named_scope("attn"):
                              o_ps = pso.tile([128, CH], F32, tag="o")
                              sum_ps = psum_p.tile([1, CH], F32, tag="sum")
                              for i, ki in enumerate(kis):
                                  first, last = (i == 0), (i == len(kis) - 1)
                                  s_ps = pss.tile([128, CH], F32, tag="s")
                                  nc.tensor.matmul(
                                      s_ps,
                                      k_full[:, 128 * ki:128 * (ki + 1)],
                                      qfin[h], start=True, stop=True)
                                  delta = CH * sc - 128 * ki
                                  j = None
                                  if delta <= 0:
                                      j = -delta // 128
                                      nc.vector.tensor_add(
                                          s_ps[:, 128 * j:128 * (j + 1)],
                                          s_ps[:, 128 * j:128 * (j + 1)], mc)
                                  elif delta >= 640:
                                      j = (1024 - delta) // 128
                                      nc.vector.tensor_add(
                                          s_ps[:, 128 * j:128 * (j + 1)],
                                          s_ps[:, 128 * j:128 * (j + 1)], mw)
                                  pr = prp.tile([128, CH], F32R, tag="pr")
                                  nc.scalar.activation(pr, s_ps, EXP)
                                  if delta <= 0 and j is not None and j > 0:
                                      nc.gpsimd.memset(
                                          pr[:, :128 * j].bitcast(F32), 0.0)
                                  if delta >= 640 and j is not None and j < 3:
                                      nc.gpsimd.memset(
                                          pr[:, 128 * (j + 1):].bitcast(F32),
                                          0.0)
                                  nc.tensor.matmul(sum_ps, on128, pr,
                                                   start=first, stop=last)
                                  nc.tensor.matmul(o_ps, v_full[:, ki, :], pr,
                                                   start=first, stop=last)
                              # tail: free o_ps early, then normalize.
                              # 1/sum applied as (1/sqrt(sum))^2 via two
                              # multiplies with the broadcast rsqrt row.
                              a_un = wrk.tile([128, CH], F32, tag="a_un")
                              nc.vector.tensor_copy(a_un, o_ps)
                              rsq = wrk.tile([1, CH], F32R, tag="rsq")
                              nc.scalar.activation(rsq, sum_ps, RSQRT)
                              bc = paux.tile([128, CH], F32, tag="s")
                              nc.tensor.matmul(bc, oncol, rsq,
                                               start=True, stop=True)
                              a_t1 = wrk.tile([128, CH], F32, tag="a_t1")
                              nc.vector.tensor_mul(a_t1, bc, a_un)
                              a_fin = wrk.tile([128, CH], F32R, tag="a_fin")
                              nc.vector.tensor_mul(a_fin, bc, a_t1)
                              nc.sync.dma_start(
                                  scr[b][h, :, s0:s0 + CH], a_fin)
                              if DEBUG and b == 0:
                                  nc.sync.dma_start(
                                      dbg_a.ap()[h, :, s0:s0 + CH],
                                      a_fin.bitcast(F32))
                                  nc.sync.dma_start(
                                      dbg_q.ap()[h, :, s0:s0 + CH],
                                      qfin[h].bitcast(F32))

                    if DEBUG and b == 0:
                        nc.sync.dma_start(dbg_k.ap(), k_full.bitcast(F32))
                        nc.sync.dma_start(dbg_v.ap(), v_full.bitcast(F32))

            # ---------------- phase C: output projection -----------------
            with tc.tile_pool(name="wo", bufs=1) as wop, \
                 tc.tile_pool(name="at", bufs=4) as atp, \
                 tc.tile_pool(name="ostg", bufs=4) as ost, \
                 tc.tile_pool(name="ps_c", bufs=4, space="PSUM") as pc:
                wo_sb = wop.tile([128, QW // 128, H], F32R)
                nc.sync.dma_start(
                    wo_sb, wo_s.ap().rearrange("(o p) c -> p o c", p=128))
                NR = QW // 128
                for b in range(B):
                    for st in range(S // 128):
                        a_t = []
                        for r in range(NR):
                            at = atp.tile([128, 128], F32R, tag=f"at{r}")
                            nc.sync.dma_start(
                                at, scr[b][r, :, 128 * st:128 * (st + 1)])
                            a_t.append(at)
                        # rotate the psum bank between consecutive matmuls:
                        # 4 output blocks in flight, contraction (r) outer.
                        for g in range(2):
                            hcbs = range(4 * g, 4 * g + 4)
                            c_tiles = {hcb: pc.tile([128, 512], F32, tag="c",
                                                    name=f"c{hcb % 4}")
                                       for hcb in hcbs}
                            for r in range(NR):
                                for hcb in hcbs:
                                    nc.tensor.matmul(
                                        c_tiles[hcb], a_t[r],
                                        wo_sb[:, r,
                                              512 * hcb:512 * (hcb + 1)],
                                        start=(r == 0), stop=(r == NR - 1))
                            for hcb in hcbs:
                                o_sb = ost.tile([128, 512], F32, tag="ostg")
                                nc.scalar.copy(o_sb, c_tiles[hcb])
                                nc.sync.dma_start(
                                    out_part.ap()[b, 128 * st:128 * (st + 1),
                                                  512 * hcb:512 * (hcb + 1)],
                                    o_sb)

    nc.compile()
    return nc


def _host_prep(hidden_states, wq, wk, wv, wo, q_norm_w, k_norm_w):
    """Build the per-core input maps (all float32 numpy)."""
    f32 = np.float32
    hsT = np.ascontiguousarray(
        np.transpose(hidden_states.astype(f32), (0, 2, 1)))

    pos = np.arange(S, dtype=np.float64)
    inv_freq = 1.0 / (THETA ** (np.arange(0, D, 2, dtype=np.float64) / D))
    freqs = pos[:, None] * inv_freq[None, :]
    emb = np.concatenate([freqs, freqs], axis=-1)           # [S, D]
    cosT = np.ascontiguousarray(np.cos(emb).T.astype(f32))  # [D, S]
    sinT = np.ascontiguousarray(np.sin(emb).T.astype(f32))

    protT = np.zeros((128, 128), f32)
    protT[64 + np.arange(64), np.arange(64)] = -1.0
    protT[np.arange(64), 64 + np.arange(64)] = 1.0

    kd = np.arange(128)[:, None]
    qd = np.arange(128)[None, :]
    mask_c = np.where(qd >= kd, 0.0, NEG).astype(f32)
    mask_w = np.where(qd < kd, 0.0, NEG).astype(f32)

    common = {
        "hsT": hsT,
        "cosT": cosT,
        "sinT": sinT,
        "ones128": np.ones((128, 1), f32),
        "ones_col": np.ones((1, 128), f32),
        "protT": protT,
        "idn": np.eye(128, dtype=f32),
        "mask_c": mask_c,
        "mask_w": mask_w,
        "wrow_q": (q_norm_w.astype(f32) / np.sqrt(D)).reshape(1, D),
        "wrow_k": k_norm_w.astype(f32).reshape(1, D),
    }
    in_maps = []
    for c in range(NCORES):
        m = dict(common)
        m["wq_s"] = np.ascontiguousarray(wq[:, QW * c:QW * (c + 1)]).astype(f32)
        m["wk_s"] = np.ascontiguousarray(wk[:, D * c:D * (c + 1)]).astype(f32)
        m["wv_s"] = np.ascontiguousarray(wv[:, D * c:D * (c + 1)]).astype(f32)
        m["wo_s"] = np.ascontiguousarray(wo[QW * c:QW * (c + 1), :]).astype(f32)
        in_maps.append(m)
    return in_maps


def kernel(hidden_states, wq, wk, wv, wo, q_norm_w, k_norm_w,
           _trace=False, _return_results=False):
    from concourse import bass_utils

    hidden_states = np.asarray(hidden_states)
    wq, wk, wv, wo = (np.asarray(a) for a in (wq, wk, wv, wo))
    q_norm_w, k_norm_w = np.asarray(q_norm_w), np.asarray(k_norm_w)

    if "nc" not in _CACHE:
        _CACHE["nc"] = _build()
    nc = _CACHE["nc"]

    in_maps = _host_prep(hidden_states, wq, wk, wv, wo, q_norm_w, k_norm_w)
    res = bass_utils.run_bass_kernel_spmd(
        nc, in_maps, core_ids=list(range(NCORES)), trace=_trace)

    out = np.zeros((B, S, H), np.float32)
    for c in range(NCORES):
        out += res.results[c]["out_part"]
    if _return_results:
        return out, res
    return out



# revision 5
# speedup vs baseline: 1.1948x; 1.1948x over previous
"""Exaone4 attention kernel for 8 Trainium2 NeuronCores.

Sharding: tensor-parallel over heads (TP=8). Core i owns query heads
4i..4i+3 and kv head i (one GQA group), processes both batch elements,
and computes a row-parallel partial of the output projection; the host
sums the 8 partials.

Pipeline (fp16 wire dtypes, fp32 PSUM/stats):
  per batch b:
    QKV: single pass over the contraction (6 psum banks: q0..q3,k,v),
         epilogue does RMSNorm via ones-matmul + broadcast-rsqrt and
         RoPE via partition-offset DVE ops (sign folded into the sin
         table, norm weights into per-partition scalars). V transposed
         to [tok, d] tiles with DMA xbar transposes.
    attention: per (chunk, head): scores/PV matmuls depth-4 pipelined;
         exp on ACT (bias -10, sliced to the unmasked region); softmax
         denominator accumulated on DVE and reduced with one
         ones-matmul; 1/sum via DVE fast reciprocal. Outputs stay in
         SBUF (no DRAM scratch).
  out-projection: reads attention outputs straight from SBUF, writes
         fp16 partials; host sums in fp32.

Shapes (hardcoded): B=2, S=2048, H=4096, NH=32, NKV=8, D=128,
WINDOW=1024, eps=1e-5, theta=10000.
"""

import os
import sys

for _p in ("/opt/trn_rl_repo",):
    if _p not in sys.path and os.path.isdir(_p):
        sys.path.insert(0, _p)

import numpy as np

B, S, H = 2, 2048, 4096
NH, NKV, D = 32, 8, 128
WINDOW = 1024
EPS = 1e-5
THETA = 10000.0

NCORES = 8
HPC = NH // NCORES          # query heads per core = 4
QW = HPC * D                # q-proj cols per core = 512
CH = 512                    # sequence chunk
NSC = S // CH               # 4 chunks
HC = H // 128               # 32 contraction chunks
NEG = -1.0e30
EXPB = -10.0                # exp bias so fp16 probs never overflow

_CACHE = {}


def _build():
    import concourse.bass as bass
    import concourse.tile as tile
    from concourse import mybir, bacc

    F32 = mybir.dt.float32
    F16 = mybir.dt.float16
    EXP = mybir.ActivationFunctionType.Exp
    SQUARE = mybir.ActivationFunctionType.Square
    RSQRT = mybir.ActivationFunctionType.Abs_reciprocal_sqrt
    MULT = mybir.AluOpType.mult

    nc = bacc.Bacc("TRN2", target_bir_lowering=False, debug=False)

    hsT = nc.dram_tensor("hsT", [B, H, S], F16, kind="ExternalInput")
    wq_s = nc.dram_tensor("wq_s", [H, QW], F16, kind="ExternalInput")
    wk_s = nc.dram_tensor("wk_s", [H, D], F16, kind="ExternalInput")
    wv_s = nc.dram_tensor("wv_s", [H, D], F16, kind="ExternalInput")
    wo_s = nc.dram_tensor("wo_s", [QW, H], F16, kind="ExternalInput")
    cosT = nc.dram_tensor("cosT", [D, S], F16, kind="ExternalInput")
    sinT = nc.dram_tensor("sinT", [D, S], F16, kind="ExternalInput")
    wqcos_d = nc.dram_tensor("wqcos", [D, 1], F32, kind="ExternalInput")
    wqsin_d = nc.dram_tensor("wqsin", [D, 1], F32, kind="ExternalInput")
    wkcos_d = nc.dram_tensor("wkcos", [D, 1], F32, kind="ExternalInput")
    wksin_d = nc.dram_tensor("wksin", [D, 1], F32, kind="ExternalInput")
    mask_c = nc.dram_tensor("mask_c", [128, 128], F32, kind="ExternalInput")
    mask_w = nc.dram_tensor("mask_w", [128, 128], F32, kind="ExternalInput")
    out_part = nc.dram_tensor("out_part", [B, S, H], F16, kind="ExternalOutput")

    with tile.TileContext(nc) as tc, \
         nc.allow_low_precision(reason="deliberate fp16 matmul pipeline"):
        with tc.tile_pool(name="consts", bufs=1) as consts:
            cos_sb = consts.tile([D, S], F16)
            nc.sync.dma_start(cos_sb, cosT.ap())
            sin_sb = consts.tile([D, S], F16)
            nc.sync.dma_start(sin_sb, sinT.ap())
            wqcos = consts.tile([D, 1], F32)
            nc.sync.dma_start(wqcos, wqcos_d.ap())
            wqsin = consts.tile([D, 1], F32)
            nc.sync.dma_start(wqsin, wqsin_d.ap())
            wkcos = consts.tile([D, 1], F32)
            nc.sync.dma_start(wkcos, wkcos_d.ap())
            wksin = consts.tile([D, 1], F32)
            nc.sync.dma_start(wksin, wksin_d.ap())
            mc = consts.tile([128, 128], F32)
            nc.sync.dma_start(mc, mask_c.ap())
            mw = consts.tile([128, 128], F32)
            nc.sync.dma_start(mw, mask_w.ap())
            ones_k = consts.tile([128, 128], F16)
            nc.vector.memset(ones_k, 1.0)
            expb = consts.tile([128, 1], F32)
            nc.vector.memset(expb, EXPB)
            bias_q = consts.tile([128, 1], F32)
            nc.vector.memset(bias_q, float(D) * EPS)
            bias_k = consts.tile([128, 1], F32)
            nc.vector.memset(bias_k, EPS)

            wq_sb = consts.tile([128, HC, QW], F16)
            nc.sync.dma_start(
                wq_sb, wq_s.ap().rearrange("(o p) c -> p o c", p=128))
            wk_sb = consts.tile([128, HC, D], F16)
            nc.sync.dma_start(
                wk_sb, wk_s.ap().rearrange("(o p) c -> p o c", p=128))
            wv_sb = consts.tile([128, HC, D], F16)
            nc.sync.dma_start(
                wv_sb, wv_s.ap().rearrange("(o p) c -> p o c", p=128))
            wo_sb = consts.tile([128, QW // 128, H], F16)
            nc.sync.dma_start(
                wo_sb, wo_s.ap().rearrange("(o p) c -> p o c", p=128))

            # attention outputs, SBUF-resident until the out-projection
            scr = consts.tile([128, B, HPC, S], F16)
            # per-batch q/k/v (reused across b)
            qf = consts.tile([128, HPC, S], F16)
            k_full = consts.tile([128, S], F16)
            v_full = consts.tile([128, S // 128, 128], F16)

            # per-group norm blocks: (psum slot, kind, head idx)
            grp_norm = [
                [(0, "q", 0), (1, "q", 1), (2, "q", 2)],
                [(0, "q", 3), (1, "k", 0)],          # slot 2 of grp1 is V
            ]

            for b in range(B):
                # ---------------- QKV + norm + rope ----------------------
                # Two 3-block groups per chunk (q0q1q2 | q3,k,v), psum
                # bufs=2 so a group's epilogue overlaps the next group's
                # matmul stream. The epilogue's PE ops (ssq matmuls) are
                # deferred into the next group's stream so PE never
                # stalls on the ACT square pass.
                with tc.tile_pool(name="hs", bufs=4) as hsp, \
                     tc.tile_pool(name="epi", bufs=2) as epi, \
                     tc.tile_pool(name="ps_qkv", bufs=2, space="PSUM") as pq, \
                     tc.tile_pool(name="ps_aux", bufs=2, space="PSUM") as pa:

                    def epilogue_rest(sc, grp, qkv_ps, sqs):
                        """ssq matmuls + rsqrt + normalize + rope for one
                        group (everything after the ACT squares)."""
                        s0 = CH * sc
                        ssqs, wtils = {}, {}
                        for i, (blk, kind, hd) in enumerate(grp_norm[grp]):
                            ssq = pa.tile([128, CH], F32, tag="aux")
                            nc.tensor.matmul(ssq, ones_k, sqs[i],
                                             start=True, stop=True)
                            ssqs[i] = ssq
                        for i, (blk, kind, hd) in enumerate(grp_norm[grp]):
                            # q: rsqrt(ssq + D*eps) = rsqrt(mean+eps)/sqrt(D)
                            # (folds the 1/sqrt(D) score scale into q)
                            wtil = epi.tile([128, CH], F16, tag=f"wtil{i}")
                            if kind == "q":
                                nc.scalar.activation(wtil, ssqs[i], RSQRT,
                                                     bias=bias_q, scale=1.0)
                            else:
                                nc.scalar.activation(wtil, ssqs[i], RSQRT,
                                                     bias=bias_k,
                                                     scale=1.0 / D)
                            wtils[i] = wtil
                        for i, (blk, kind, hd) in enumerate(grp_norm[grp]):
                            qhat = epi.tile([128, CH], F16, tag=f"qhat{i}")
                            nc.vector.tensor_mul(qhat, qkv_ps[:, blk, :],
                                                 wtils[i])
                            wcos = wqcos if kind == "q" else wkcos
                            wsin = wqsin if kind == "q" else wksin
                            dst = (qf[:, hd, s0:s0 + CH] if kind == "q"
                                   else k_full[:, s0:s0 + CH])
                            t1 = epi.tile([128, CH], F16, tag=f"t1_{i}")
                            nc.vector.scalar_tensor_tensor(
                                t1, qhat, wcos, cos_sb[:, s0:s0 + CH],
                                op0=MULT, op1=MULT)
                            # in0/in1 must share a base partition (compiler
                            # constraint); sin[d] == sin[d^64] since the
                            # rope table is [freqs, freqs], so slicing sin
                            # at in0's base yields the right values.
                            t2 = epi.tile([128, CH], F16, tag=f"t2_{i}")
                            nc.vector.scalar_tensor_tensor(
                                t2[0:64, :], qhat[64:128, :], wsin[64:128, :],
                                sin_sb[64:128, s0:s0 + CH],
                                op0=MULT, op1=MULT)
                            nc.vector.scalar_tensor_tensor(
                                t2[64:128, :], qhat[0:64, :], wsin[0:64, :],
                                sin_sb[0:64, s0:s0 + CH],
                                op0=MULT, op1=MULT)
                            nc.vector.tensor_add(dst, t1, t2)

                    pending = None
                    for sc in range(NSC):
                        s0 = CH * sc
                        for grp in range(2):
                            qkv_ps = pq.tile([128, 3, CH], F32, tag="qkv")
                            for hc in range(HC):
                                if hc == 6 and pending is not None:
                                    epilogue_rest(*pending)
                                    pending = None
                                ht = hsp.tile([128, CH], F16, tag="ht")
                                nc.sync.dma_start(
                                    ht, hsT.ap()[b, 128 * hc:128 * (hc + 1),
                                                 s0:s0 + CH])
                                for bi in range(3):
                                    blk = 3 * grp + bi
                                    if blk < HPC:
                                        lhs = wq_sb[:, hc,
                                                    128 * blk:128 * (blk + 1)]
                                    elif blk == HPC:
                                        lhs = wk_sb[:, hc, :]
                                    else:
                                        lhs = wv_sb[:, hc, :]
                                    nc.tensor.matmul(
                                        qkv_ps[:, bi, :], lhs, ht,
                                        start=(hc == 0), stop=(hc == HC - 1))

                            if grp == 1:
                                # V: cast + DMA xbar transpose to [tok, d]
                                v16 = epi.tile([128, CH], F16, tag="v16")
                                nc.scalar.copy(v16, qkv_ps[:, 2, :])
                                for j in range(CH // 128):
                                    nc.sync.dma_start_transpose(
                                        v_full[:, 4 * sc + j, :],
                                        v16[:, 128 * j:128 * (j + 1)])
                            # ACT squares inline; the rest is deferred into
                            # the next group's matmul stream.
                            sqs = {}
                            for i, (blk, kind, hd) in enumerate(grp_norm[grp]):
                                sq = epi.tile([128, CH], F16, tag=f"sq{i}")
                                nc.scalar.activation(sq, qkv_ps[:, blk, :],
                                                     SQUARE)
                                sqs[i] = sq
                            pending = (sc, grp, qkv_ps, sqs)
                    # last group's epilogue before the pool closes
                    epilogue_rest(*pending)
                    pending = None

                # ---------------- attention for batch b ------------------
                with tc.tile_pool(name="pr", bufs=4) as prp, \
                     tc.tile_pool(name="acc", bufs=2) as accp, \
                     tc.tile_pool(name="tail", bufs=2) as tlp, \
                     tc.tile_pool(name="ps_s", bufs=4, space="PSUM") as pss, \
                     tc.tile_pool(name="ps_o", bufs=2, space="PSUM") as pso, \
                     tc.tile_pool(name="ps_m", bufs=2, space="PSUM") as psm:
                    LOOK = 4
                    for sc in range(NSC):
                        s0 = CH * sc
                        kis = list(range(max(0, 4 * sc - 8), 4 * sc + 4))
                        n = len(kis)
                        for h in range(HPC):
                            o_ps = pso.tile([128, CH], F32, tag="o")
                            pracc = accp.tile([128, CH], F16, tag="acc")
                            prs = {}

                            def score(i):
                                ki = kis[i]
                                s_ps = pss.tile([128, CH], F32, tag="s")
                                nc.tensor.matmul(
                                    s_ps,
                                    k_full[:, 128 * ki:128 * (ki + 1)],
                                    qf[:, h, s0:s0 + CH],
                                    start=True, stop=True)
                                delta = CH * sc - 128 * ki
                                lo, hi = 0, CH
                                if delta <= 0:
                                    j = -delta // 128
                                    nc.vector.tensor_add(
                                        s_ps[:, 128 * j:128 * (j + 1)],
                                        s_ps[:, 128 * j:128 * (j + 1)], mc)
                                    lo = 128 * j
                                elif delta >= 640:
                                    j = (1024 - delta) // 128
                                    nc.vector.tensor_add(
                                        s_ps[:, 128 * j:128 * (j + 1)],
                                        s_ps[:, 128 * j:128 * (j + 1)], mw)
                                    hi = 128 * (j + 1)
                                pr = prp.tile([128, CH], F16, tag="pr")
                                nc.scalar.activation(pr[:, lo:hi],
                                                     s_ps[:, lo:hi], EXP,
                                                     bias=expb)
                                if lo > 0:
                                    nc.gpsimd.memset(pr[:, :lo], 0.0)
                                if hi < CH:
                                    nc.gpsimd.memset(pr[:, hi:], 0.0)
                                if i == 0:
                                    nc.vector.tensor_copy(pracc, pr)
                                else:
                                    nc.vector.tensor_add(pracc, pracc, pr)
                                prs[i] = pr

                            def pv(i):
                                nc.tensor.matmul(
                                    o_ps, v_full[:, kis[i], :], prs[i],
                                    start=(i == 0), stop=(i == n - 1))

                            for i in range(min(LOOK, n)):
                                score(i)
                            for i in range(n):
                                pv(i)
                                if i + LOOK < n:
                                    score(i + LOOK)

                            # tail: sum (broadcast over partitions via
                            # all-ones lhsT), clamp, fast reciprocal,
                            # normalize into SBUF-resident scr.
                            sum_ps = psm.tile([128, CH], F32, tag="sum")
                            nc.tensor.matmul(sum_ps, ones_k, pracc,
                                             start=True, stop=True)
                            cl = tlp.tile([128, CH], F32, tag="cl")
                            nc.vector.tensor_scalar_max(cl, sum_ps, 1e-9)
                            rq = tlp.tile([128, CH], F32, tag="rq")
                            nc.vector.reciprocal_approx_fast(rq, cl)
                            nc.vector.tensor_mul(
                                scr[:, b, h, s0:s0 + CH], o_ps, rq)

            # ---------------- output projection ----------------------
            with tc.tile_pool(name="ostg", bufs=4) as ost, \
                 tc.tile_pool(name="ps_c", bufs=1, space="PSUM") as pc:
                NR = QW // 128
                for b in range(B):
                    for st in range(S // 128):
                        for g in range(2):
                            hcbs = range(4 * g, 4 * g + 4)
                            c_tiles = {hcb: pc.tile([128, 512], F32,
                                                    tag=f"c{hcb}",
                                                    name=f"c{hcb}")
                                       for hcb in hcbs}
                            for r in range(NR):
                                a_t = scr[:, b, r, 128 * st:128 * (st + 1)]
                                for hcb in hcbs:
                                    nc.tensor.matmul(
                                        c_tiles[hcb], a_t,
                                        wo_sb[:, r,
                                              512 * hcb:512 * (hcb + 1)],
                                        start=(r == 0), stop=(r == NR - 1))
                            for hcb in hcbs:
                                o_sb = ost.tile([128, 512], F16, tag="ostg")
                                nc.scalar.copy(o_sb, c_tiles[hcb])
                                nc.sync.dma_start(
                                    out_part.ap()[b, 128 * st:128 * (st + 1),
                                                  512 * hcb:512 * (hcb + 1)],
                                    o_sb)

    nc.compile()
    return nc


def _host_prep(hidden_states, wq, wk, wv, wo, q_norm_w, k_norm_w):
    """Build the per-core input maps (fp16 wire dtypes)."""
    f16 = np.float16
    f32 = np.float32
    hsT = np.ascontiguousarray(
        np.transpose(hidden_states, (0, 2, 1))).astype(f16)

    pos = np.arange(S, dtype=np.float64)
    inv_freq = 1.0 / (THETA ** (np.arange(0, D // 2, dtype=np.float64)
                                / (D // 2)))
    freqs = pos[None, :] * inv_freq[:, None]            # [D/2, S]
    emb = np.concatenate([freqs, freqs], axis=0)        # [D, S]
    cosT = np.cos(emb).astype(f16)
    sinT = np.sin(emb).astype(f16)

    # norm weights folded into per-partition rope scalars; the sign of
    # rotate_half folded into wsin (negative for output partitions >= 64,
    # which read input partitions < 64... sign indexed by input partition:
    # wsin[p] multiplies qhat[p] feeding output partition (p+64)%128.
    qw = q_norm_w.astype(f32)
    kw = k_norm_w.astype(f32)
    sgn = np.where(np.arange(D) < 64, 1.0, -1.0).astype(f32)
    wqcos = qw.reshape(D, 1)
    wqsin = (qw * sgn).reshape(D, 1)
    wkcos = kw.reshape(D, 1)
    wksin = (kw * sgn).reshape(D, 1)

    kd = np.arange(128)[:, None]
    qd = np.arange(128)[None, :]
    mask_c = np.where(qd >= kd, 0.0, NEG).astype(f32)
    mask_w = np.where(qd < kd, 0.0, NEG).astype(f32)

    common = {
        "hsT": hsT,
        "cosT": cosT,
        "sinT": sinT,
        "wqcos": wqcos,
        "wqsin": wqsin,
        "wkcos": wkcos,
        "wksin": wksin,
        "mask_c": mask_c,
        "mask_w": mask_w,
    }
    in_maps = []
    for c in range(NCORES):
        m = dict(common)
        m["wq_s"] = np.ascontiguousarray(wq[:, QW * c:QW * (c + 1)]).astype(f16)
        m["wk_s"] = np.ascontiguousarray(wk[:, D * c:D * (c + 1)]).astype(f16)
        m["wv_s"] = np.ascontiguousarray(wv[:, D * c:D * (c + 1)]).astype(f16)
        m["wo_s"] = np.ascontiguousarray(wo[QW * c:QW * (c + 1), :]).astype(f16)
        in_maps.append(m)
    return in_maps


def kernel(hidden_states, wq, wk, wv, wo, q_norm_w, k_norm_w,
           _trace=False, _return_results=False):
    from concourse import bass_utils

    hidden_states = np.asarray(hidden_states)
    wq, wk, wv, wo = (np.asarray(a) for a in (wq, wk, wv, wo))
    q_norm_w, k_norm_w = np.asarray(q_norm_w), np.asarray(k_norm_w)

    if "nc" not in _CACHE:
        _CACHE["nc"] = _build()
    nc = _CACHE["nc"]

    in_maps = _host_prep(hidden_states, wq, wk, wv, wo, q_norm_w, k_norm_w)
    res = bass_utils.run_bass_kernel_spmd(
        nc, in_maps, core_ids=list(range(NCORES)), trace=_trace)

    out = np.zeros((B, S, H), np.float32)
    for c in range(NCORES):
        out += res.results[c]["out_part"].astype(np.float32)
    if _return_results:
        return out, res
    return out


# revision 8
# speedup vs baseline: 1.6538x; 1.3842x over previous
"""Exaone4 attention kernel for 8 Trainium2 NeuronCores.

Sharding: tensor-parallel over heads (TP=8). Core i owns query heads
4i..4i+3 and kv head i (one GQA group), processes both batch elements,
and computes a row-parallel partial of the output projection; the host
sums the 8 partials.

Pipeline (fp16 wire dtypes, fp32 PSUM/stats):
  per batch b:
    QKV: single pass over the contraction (6 psum banks: q0..q3,k,v),
         epilogue does RMSNorm via ones-matmul + broadcast-rsqrt and
         RoPE via partition-offset DVE ops (sign folded into the sin
         table, norm weights into per-partition scalars). V transposed
         to [tok, d] tiles with DMA xbar transposes.
    attention: per (chunk, head): scores/PV matmuls depth-4 pipelined;
         exp on ACT (bias -10, sliced to the unmasked region); softmax
         denominator accumulated on DVE and reduced with one
         ones-matmul; 1/sum via DVE fast reciprocal. Outputs stay in
         SBUF (no DRAM scratch).
  out-projection: reads attention outputs straight from SBUF, writes
         fp16 partials; host sums in fp32.

Shapes (hardcoded): B=2, S=2048, H=4096, NH=32, NKV=8, D=128,
WINDOW=1024, eps=1e-5, theta=10000.
"""

import os
import sys

for _p in ("/opt/trn_rl_repo",):
    if _p not in sys.path and os.path.isdir(_p):
        sys.path.insert(0, _p)

import numpy as np

B, S, H = 2, 2048, 4096
NH, NKV, D = 32, 8, 128
WINDOW = 1024
EPS = 1e-5
THETA = 10000.0

NCORES = 8
HPC = NH // NCORES          # query heads per core = 4
QW = HPC * D                # q-proj cols per core = 512
CH = 512                    # sequence chunk
NSC = S // CH               # 4 chunks
HC = H // 128               # 32 contraction chunks
NEG = -1.0e30
EXPB = -10.0                # exp bias so fp16 probs never overflow

_CACHE = {}


def _build():
    import concourse.bass as bass
    import concourse.tile as tile
    from concourse import mybir, bacc

    F32 = mybir.dt.float32
    F16 = mybir.dt.float16
    EXP = mybir.ActivationFunctionType.Exp
    SQUARE = mybir.ActivationFunctionType.Square
    RSQRT = mybir.ActivationFunctionType.Abs_reciprocal_sqrt
    MULT = mybir.AluOpType.mult

    nc = bacc.Bacc("TRN2", target_bir_lowering=False, debug=False)

    hsT = nc.dram_tensor("hsT", [B, H, S], F16, kind="ExternalInput")
    wq_s = nc.dram_tensor("wq_s", [H, QW], F16, kind="ExternalInput")
    wk_s = nc.dram_tensor("wk_s", [H, D], F16, kind="ExternalInput")
    wv_s = nc.dram_tensor("wv_s", [H, D], F16, kind="ExternalInput")
    wo_s = nc.dram_tensor("wo_s", [QW, H], F16, kind="ExternalInput")
    cosT = nc.dram_tensor("cosT", [D, S], F16, kind="ExternalInput")
    sinT = nc.dram_tensor("sinT", [D, S], F16, kind="ExternalInput")
    wqcos_d = nc.dram_tensor("wqcos", [D, 1], F32, kind="ExternalInput")
    wqsin_d = nc.dram_tensor("wqsin", [D, 1], F32, kind="ExternalInput")
    wkcos_d = nc.dram_tensor("wkcos", [D, 1], F32, kind="ExternalInput")
    wksin_d = nc.dram_tensor("wksin", [D, 1], F32, kind="ExternalInput")
    mask_c = nc.dram_tensor("mask_c", [128, 128], F32, kind="ExternalInput")
    mask_w = nc.dram_tensor("mask_w", [128, 128], F32, kind="ExternalInput")
    out_part = nc.dram_tensor("out_part", [B, S, H], F16, kind="ExternalOutput")

    with tile.TileContext(nc) as tc, \
         nc.allow_low_precision(reason="deliberate fp16 matmul pipeline"):
        with tc.tile_pool(name="consts", bufs=1) as consts:
            cos_sb = consts.tile([D, S], F16)
            nc.sync.dma_start(cos_sb, cosT.ap())
            sin_sb = consts.tile([D, S], F16)
            nc.sync.dma_start(sin_sb, sinT.ap())
            wqcos = consts.tile([D, 1], F32)
            nc.sync.dma_start(wqcos, wqcos_d.ap())
            wqsin = consts.tile([D, 1], F32)
            nc.sync.dma_start(wqsin, wqsin_d.ap())
            wkcos = consts.tile([D, 1], F32)
            nc.sync.dma_start(wkcos, wkcos_d.ap())
            wksin = consts.tile([D, 1], F32)
            nc.sync.dma_start(wksin, wksin_d.ap())
            mc = consts.tile([128, 128], F32)
            nc.sync.dma_start(mc, mask_c.ap())
            mw = consts.tile([128, 128], F32)
            nc.sync.dma_start(mw, mask_w.ap())
            ones_k = consts.tile([128, 128], F16)
            nc.vector.memset(ones_k, 1.0)
            expb = consts.tile([128, 1], F32)
            nc.vector.memset(expb, EXPB)
            bias_q = consts.tile([128, 1], F32)
            nc.vector.memset(bias_q, float(D) * EPS)
            bias_k = consts.tile([128, 1], F32)
            nc.vector.memset(bias_k, EPS)

            wq_sb = consts.tile([128, HC, QW], F16)
            nc.sync.dma_start(
                wq_sb, wq_s.ap().rearrange("(o p) c -> p o c", p=128))
            wk_sb = consts.tile([128, HC, D], F16)
            nc.sync.dma_start(
                wk_sb, wk_s.ap().rearrange("(o p) c -> p o c", p=128))
            wv_sb = consts.tile([128, HC, D], F16)
            nc.sync.dma_start(
                wv_sb, wv_s.ap().rearrange("(o p) c -> p o c", p=128))
            wo_sb = consts.tile([128, QW // 128, H], F16)
            nc.sync.dma_start(
                wo_sb, wo_s.ap().rearrange("(o p) c -> p o c", p=128))

            # attention outputs, SBUF-resident until the out-projection
            scr = consts.tile([128, B, HPC, S], F16)
            # per-batch q/k/v (reused across b)
            qf = consts.tile([128, HPC, S], F16)
            k_full = consts.tile([128, S], F16)
            v_full = consts.tile([128, S // 128, 128], F16)

            # per-group norm blocks: (psum slot, kind, head idx)
            grp_norm = [
                [(0, "q", 0), (1, "q", 1), (2, "q", 2)],
                [(0, "q", 3), (1, "k", 0)],          # slot 2 of grp1 is V
            ]

            for b in range(B):
                # ---------------- QKV + norm + rope ----------------------
                # Two 3-block groups per chunk (q0q1q2 | q3,k,v), psum
                # bufs=2 so a group's epilogue overlaps the next group's
                # matmul stream. The epilogue's PE ops (ssq matmuls) are
                # deferred into the next group's stream so PE never
                # stalls on the ACT square pass.
                with tc.tile_pool(name="hs", bufs=8) as hsp, \
                     tc.tile_pool(name="epi", bufs=2) as epi, \
                     tc.tile_pool(name="ps_qkv", bufs=2, space="PSUM") as pq, \
                     tc.tile_pool(name="ps_aux", bufs=2, space="PSUM") as pa:

                    def epilogue_rest(sc, grp, qkv_ps, sqs):
                        """ssq matmuls + rsqrt + normalize + rope for one
                        group (everything after the ACT squares)."""
                        s0 = CH * sc
                        ssqs, wtils = {}, {}
                        for i, (blk, kind, hd) in enumerate(grp_norm[grp]):
                            ssq = pa.tile([128, CH], F32, tag="aux")
                            nc.tensor.matmul(ssq, ones_k, sqs[i],
                                             start=True, stop=True)
                            ssqs[i] = ssq
                        for i, (blk, kind, hd) in enumerate(grp_norm[grp]):
                            # q: rsqrt(ssq + D*eps) = rsqrt(mean+eps)/sqrt(D)
                            # (folds the 1/sqrt(D) score scale into q)
                            wtil = epi.tile([128, CH], F16, tag=f"wtil{i}")
                            if kind == "q":
                                nc.scalar.activation(wtil, ssqs[i], RSQRT,
                                                     bias=bias_q, scale=1.0)
                            else:
                                nc.scalar.activation(wtil, ssqs[i], RSQRT,
                                                     bias=bias_k,
                                                     scale=1.0 / D)
                            wtils[i] = wtil
                        for i, (blk, kind, hd) in enumerate(grp_norm[grp]):
                            qhat = epi.tile([128, CH], F16, tag=f"qhat{i}")
                            nc.vector.tensor_mul(qhat, qkv_ps[:, blk, :],
                                                 wtils[i])
                            wcos = wqcos if kind == "q" else wkcos
                            wsin = wqsin if kind == "q" else wksin
                            dst = (qf[:, hd, s0:s0 + CH] if kind == "q"
                                   else k_full[:, s0:s0 + CH])
                            t1 = epi.tile([128, CH], F16, tag=f"t1_{i}")
                            nc.vector.scalar_tensor_tensor(
                                t1, qhat, wcos, cos_sb[:, s0:s0 + CH],
                                op0=MULT, op1=MULT)
                            # in0/in1 must share a base partition (compiler
                            # constraint); sin[d] == sin[d^64] since the
                            # rope table is [freqs, freqs], so slicing sin
                            # at in0's base yields the right values.
                            t2 = epi.tile([128, CH], F16, tag=f"t2_{i}")
                            nc.vector.scalar_tensor_tensor(
                                t2[0:64, :], qhat[64:128, :], wsin[64:128, :],
                                sin_sb[64:128, s0:s0 + CH],
                                op0=MULT, op1=MULT)
                            nc.vector.scalar_tensor_tensor(
                                t2[64:128, :], qhat[0:64, :], wsin[0:64, :],
                                sin_sb[0:64, s0:s0 + CH],
                                op0=MULT, op1=MULT)
                            nc.vector.tensor_add(dst, t1, t2)

                    pending = None
                    for sc in range(NSC):
                        s0 = CH * sc
                        for grp in range(2):
                            qkv_ps = pq.tile([128, 3, CH], F32, tag="qkv")
                            for hp in range(HC // 2):
                                if hp == 3 and pending is not None:
                                    epilogue_rest(*pending)
                                    pending = None
                                ht = hsp.tile([128, 2, CH], F16, tag="ht")
                                nc.sync.dma_start(
                                    ht,
                                    hsT.ap()[b, 256 * hp:256 * (hp + 1),
                                             s0:s0 + CH].rearrange(
                                        "(o p) c -> p o c", p=128))
                                for sub in range(2):
                                    hc = 2 * hp + sub
                                    for bi in range(3):
                                        blk = 3 * grp + bi
                                        if blk < HPC:
                                            lhs = wq_sb[:, hc,
                                                        128 * blk:
                                                        128 * (blk + 1)]
                                        elif blk == HPC:
                                            lhs = wk_sb[:, hc, :]
                                        else:
                                            lhs = wv_sb[:, hc, :]
                                        nc.tensor.matmul(
                                            qkv_ps[:, bi, :], lhs,
                                            ht[:, sub, :],
                                            start=(hc == 0),
                                            stop=(hc == HC - 1))

                            if grp == 1:
                                # V: cast + DMA xbar transpose to [tok, d]
                                v16 = epi.tile([128, CH], F16, tag="v16")
                                nc.scalar.copy(v16, qkv_ps[:, 2, :])
                                for j in range(CH // 128):
                                    nc.sync.dma_start_transpose(
                                        v_full[:, 4 * sc + j, :],
                                        v16[:, 128 * j:128 * (j + 1)])
                            # ACT squares inline; the rest is deferred into
                            # the next group's matmul stream.
                            sqs = {}
                            for i, (blk, kind, hd) in enumerate(grp_norm[grp]):
                                sq = epi.tile([128, CH], F16, tag=f"sq{i}")
                                nc.scalar.activation(sq, qkv_ps[:, blk, :],
                                                     SQUARE)
                                sqs[i] = sq
                            pending = (sc, grp, qkv_ps, sqs)
                    # last group's epilogue before the pool closes
                    epilogue_rest(*pending)
                    pending = None

                # ---------------- attention for batch b ------------------
                # Emission order is engine-order: on DVE the mask adds are
                # kept ahead of the pracc accumulates (else exp_{i+1} chains
                # behind acc_i and the loop serializes), and each head's
                # tail chain (sum/clamp/recip/normalize) is deferred into
                # the next head's early stream.
                with tc.tile_pool(name="pr", bufs=6) as prp, \
                     tc.tile_pool(name="acc", bufs=2) as accp, \
                     tc.tile_pool(name="tail", bufs=2) as tlp, \
                     tc.tile_pool(name="ps_s", bufs=4, space="PSUM") as pss, \
                     tc.tile_pool(name="ps_o", bufs=2, space="PSUM") as pso, \
                     tc.tile_pool(name="ps_m", bufs=2, space="PSUM") as psm:
                    LOOK = 4
                    pend_tail = None

                    def emit_tail(b_, h_, s0_, o_ps_, pracc_):
                        # sum of probs broadcast to all partitions via the
                        # all-ones lhsT, then clamp, fast reciprocal,
                        # normalize into SBUF-resident scr.
                        sum_ps = psm.tile([128, CH], F32, tag="sum")
                        nc.tensor.matmul(sum_ps, ones_k, pracc_,
                                         start=True, stop=True)
                        cl = tlp.tile([128, CH], F32, tag="cl")
                        nc.vector.tensor_scalar_max(cl, sum_ps, 1e-9)
                        rq = tlp.tile([128, CH], F32, tag="rq")
                        nc.vector.reciprocal_approx_fast(rq, cl)
                        nc.vector.tensor_mul(
                            scr[:, b_, h_, s0_:s0_ + CH], o_ps_, rq)

                    for sc in range(NSC):
                        s0 = CH * sc
                        kis = list(range(max(0, 4 * sc - 8), 4 * sc + 4))
                        n = len(kis)
                        for h in range(HPC):
                            o_ps = pso.tile([128, CH], F32, tag="o")
                            pracc = accp.tile([128, CH], F16, tag="acc")
                            prs = {}

                            def score(i):
                                ki = kis[i]
                                s_ps = pss.tile([128, CH], F32, tag="s")
                                nc.tensor.matmul(
                                    s_ps,
                                    k_full[:, 128 * ki:128 * (ki + 1)],
                                    qf[:, h, s0:s0 + CH],
                                    start=True, stop=True)
                                delta = CH * sc - 128 * ki
                                lo, hi = 0, CH
                                if delta <= 0:
                                    j = -delta // 128
                                    nc.vector.tensor_add(
                                        s_ps[:, 128 * j:128 * (j + 1)],
                                        s_ps[:, 128 * j:128 * (j + 1)], mc)
                                    lo = 128 * j
                                elif delta >= 640:
                                    j = (1024 - delta) // 128
                                    nc.vector.tensor_add(
                                        s_ps[:, 128 * j:128 * (j + 1)],
                                        s_ps[:, 128 * j:128 * (j + 1)], mw)
                                    hi = 128 * (j + 1)
                                pr = prp.tile([128, CH], F16, tag="pr")
                                nc.scalar.activation(pr[:, lo:hi],
                                                     s_ps[:, lo:hi], EXP,
                                                     bias=expb)
                                if lo > 0:
                                    nc.gpsimd.memset(pr[:, :lo], 0.0)
                                if hi < CH:
                                    nc.gpsimd.memset(pr[:, hi:], 0.0)
                                prs[i] = pr

                            def acc(i):
                                if i == 0:
                                    nc.vector.tensor_copy(pracc, prs[i])
                                else:
                                    nc.vector.tensor_add(pracc, pracc, prs[i])

                            def pv(i):
                                nc.tensor.matmul(
                                    o_ps, v_full[:, kis[i], :], prs[i],
                                    start=(i == 0), stop=(i == n - 1))

                            for i in range(min(LOOK, n)):
                                score(i)
                            if pend_tail is not None:
                                emit_tail(*pend_tail)
                                pend_tail = None
                            for i in range(n):
                                if i + LOOK < n:
                                    score(i + LOOK)
                                acc(i)
                                pv(i)
                            pend_tail = (b, h, s0, o_ps, pracc)
                    emit_tail(*pend_tail)
                    pend_tail = None

            # ---------------- output projection ----------------------
            with tc.tile_pool(name="ostg", bufs=4) as ost, \
                 tc.tile_pool(name="ps_c", bufs=1, space="PSUM") as pc:
                NR = QW // 128
                for b in range(B):
                    for st in range(S // 128):
                        for g in range(2):
                            hcbs = range(4 * g, 4 * g + 4)
                            c_tiles = {hcb: pc.tile([128, 512], F32,
                                                    tag=f"c{hcb}",
                                                    name=f"c{hcb}")
                                       for hcb in hcbs}
                            for r in range(NR):
                                a_t = scr[:, b, r, 128 * st:128 * (st + 1)]
                                for hcb in hcbs:
                                    nc.tensor.matmul(
                                        c_tiles[hcb], a_t,
                                        wo_sb[:, r,
                                              512 * hcb:512 * (hcb + 1)],
                                        start=(r == 0), stop=(r == NR - 1))
                            for hcb in hcbs:
                                o_sb = ost.tile([128, 512], F16, tag="ostg")
                                nc.scalar.copy(o_sb, c_tiles[hcb])
                                nc.sync.dma_start(
                                    out_part.ap()[b, 128 * st:128 * (st + 1),
                                                  512 * hcb:512 * (hcb + 1)],
                                    o_sb)

    nc.compile()
    return nc


def _host_prep(hidden_states, wq, wk, wv, wo, q_norm_w, k_norm_w):
    """Build the per-core input maps (fp16 wire dtypes)."""
    f16 = np.float16
    f32 = np.float32
    hsT = np.ascontiguousarray(
        np.transpose(hidden_states, (0, 2, 1))).astype(f16)

    pos = np.arange(S, dtype=np.float64)
    inv_freq = 1.0 / (THETA ** (np.arange(0, D // 2, dtype=np.float64)
                                / (D // 2)))
    freqs = pos[None, :] * inv_freq[:, None]            # [D/2, S]
    emb = np.concatenate([freqs, freqs], axis=0)        # [D, S]
    cosT = np.cos(emb).astype(f16)
    sinT = np.sin(emb).astype(f16)

    # norm weights folded into per-partition rope scalars; the sign of
    # rotate_half folded into wsin (negative for output partitions >= 64,
    # which read input partitions < 64... sign indexed by input partition:
    # wsin[p] multiplies qhat[p] feeding output partition (p+64)%128.
    qw = q_norm_w.astype(f32)
    kw = k_norm_w.astype(f32)
    sgn = np.where(np.arange(D) < 64, 1.0, -1.0).astype(f32)
    wqcos = qw.reshape(D, 1)
    wqsin = (qw * sgn).reshape(D, 1)
    wkcos = kw.reshape(D, 1)
    wksin = (kw * sgn).reshape(D, 1)

    kd = np.arange(128)[:, None]
    qd = np.arange(128)[None, :]
    mask_c = np.where(qd >= kd, 0.0, NEG).astype(f32)
    mask_w = np.where(qd < kd, 0.0, NEG).astype(f32)

    common = {
        "hsT": hsT,
        "cosT": cosT,
        "sinT": sinT,
        "wqcos": wqcos,
        "wqsin": wqsin,
        "wkcos": wkcos,
        "wksin": wksin,
        "mask_c": mask_c,
        "mask_w": mask_w,
    }
    in_maps = []
    for c in range(NCORES):
        m = dict(common)
        m["wq_s"] = np.ascontiguousarray(wq[:, QW * c:QW * (c + 1)]).astype(f16)
        m["wk_s"] = np.ascontiguousarray(wk[:, D * c:D * (c + 1)]).astype(f16)
        m["wv_s"] = np.ascontiguousarray(wv[:, D * c:D * (c + 1)]).astype(f16)
        m["wo_s"] = np.ascontiguousarray(wo[QW * c:QW * (c + 1), :]).astype(f16)
        in_maps.append(m)
    return in_maps


def kernel(hidden_states, wq, wk, wv, wo, q_norm_w, k_norm_w,
           _trace=False, _return_results=False):
    from concourse import bass_utils

    hidden_states = np.asarray(hidden_states)
    wq, wk, wv, wo = (np.asarray(a) for a in (wq, wk, wv, wo))
    q_norm_w, k_norm_w = np.asarray(q_norm_w), np.asarray(k_norm_w)

    if "nc" not in _CACHE:
        _CACHE["nc"] = _build()
    nc = _CACHE["nc"]

    in_maps = _host_prep(hidden_states, wq, wk, wv, wo, q_norm_w, k_norm_w)
    res = bass_utils.run_bass_kernel_spmd(
        nc, in_maps, core_ids=list(range(NCORES)), trace=_trace)

    out = np.zeros((B, S, H), np.float32)
    for c in range(NCORES):
        out += res.results[c]["out_part"].astype(np.float32)
    if _return_results:
        return out, res
    return out


# revision 23
# speedup vs baseline: 1.7393x; 1.0517x over previous
"""Exaone4 attention kernel for 8 Trainium2 NeuronCores.

Sharding: tensor-parallel over heads (TP=8). Core i owns query heads
4i..4i+3 and kv head i (one GQA group), processes both batch elements,
and computes a row-parallel partial of the output projection; the host
sums the 8 partials.

Pipeline (fp16 wire dtypes, fp32 PSUM/stats):
  per batch b:
    QKV: single pass over the contraction (6 psum banks: q0..q3,k,v),
         epilogue does RMSNorm via ones-matmul + broadcast-rsqrt and
         RoPE via partition-offset DVE ops (sign folded into the sin
         table, norm weights into per-partition scalars). V transposed
         to [tok, d] tiles with DMA xbar transposes.
    attention: per (chunk, head): scores/PV matmuls depth-4 pipelined;
         exp on ACT (bias -10, sliced to the unmasked region); softmax
         denominator accumulated on DVE and reduced with one
         ones-matmul; 1/sum via DVE fast reciprocal. Outputs stay in
         SBUF (no DRAM scratch).
  out-projection: reads attention outputs straight from SBUF, writes
         fp16 partials; host sums in fp32.

Shapes (hardcoded): B=2, S=2048, H=4096, NH=32, NKV=8, D=128,
WINDOW=1024, eps=1e-5, theta=10000.
"""

import os
import sys

for _p in ("/opt/trn_rl_repo",):
    if _p not in sys.path and os.path.isdir(_p):
        sys.path.insert(0, _p)

import numpy as np

B, S, H = 2, 2048, 4096
NH, NKV, D = 32, 8, 128
WINDOW = 1024
EPS = 1e-5
THETA = 10000.0

NCORES = 8
HPC = NH // NCORES          # query heads per core = 4
QW = HPC * D                # q-proj cols per core = 512
CH = 512                    # sequence chunk
NSC = S // CH               # 4 chunks
HC = H // 128               # 32 contraction chunks
NEG = -1.0e30
EXPB = -10.0                # exp bias so fp16 probs never overflow

_CACHE = {}


def _build():
    import concourse.bass as bass
    import concourse.tile as tile
    from concourse import mybir, bacc

    F32 = mybir.dt.float32
    F16 = mybir.dt.float16
    EXP = mybir.ActivationFunctionType.Exp
    SQUARE = mybir.ActivationFunctionType.Square
    RSQRT = mybir.ActivationFunctionType.Abs_reciprocal_sqrt
    MULT = mybir.AluOpType.mult

    nc = bacc.Bacc("TRN2", target_bir_lowering=False, debug=False)

    hsT = nc.dram_tensor("hsT", [B, H, S], F16, kind="ExternalInput")
    wq_s = nc.dram_tensor("wq_s", [H, QW], F16, kind="ExternalInput")
    wk_s = nc.dram_tensor("wk_s", [H, D], F16, kind="ExternalInput")
    wv_s = nc.dram_tensor("wv_s", [H, D], F16, kind="ExternalInput")
    wo_s = nc.dram_tensor("wo_s", [QW, H], F16, kind="ExternalInput")
    cosT = nc.dram_tensor("cosT", [D, S], F16, kind="ExternalInput")
    sinT = nc.dram_tensor("sinT", [D, S], F16, kind="ExternalInput")
    wqcos_d = nc.dram_tensor("wqcos", [D, 1], F32, kind="ExternalInput")
    wqsin_d = nc.dram_tensor("wqsin", [D, 1], F32, kind="ExternalInput")
    wkcos_d = nc.dram_tensor("wkcos", [D, 1], F32, kind="ExternalInput")
    wksin_d = nc.dram_tensor("wksin", [D, 1], F32, kind="ExternalInput")
    tri_c = nc.dram_tensor("tri_c", [128, 128], F16, kind="ExternalInput")
    tri_w = nc.dram_tensor("tri_w", [128, 128], F16, kind="ExternalInput")
    out_part = nc.dram_tensor("out_part", [B, S, H], F16, kind="ExternalOutput")

    with tile.TileContext(nc) as tc, \
         nc.allow_low_precision(reason="deliberate fp16 matmul pipeline"):
        with tc.tile_pool(name="consts", bufs=1) as consts:
            # Preamble DMA order is tuned for warmup: the sync queue holds
            # only what the first matmuls need (wq chunk 0, wk, wv) before
            # the hs tiles; everything else rides the ACT queue in
            # deadline order (wq chunks 1-3, rope tables, tris, wo).
            wq_sb = consts.tile([128, HC, QW], F16)
            nc.sync.dma_start(
                wq_sb[:, 0:8, :],
                wq_s.ap()[0:1024, :].rearrange("(o p) c -> p o c", p=128))
            wk_sb = consts.tile([128, HC, D], F16)
            nc.sync.dma_start(
                wk_sb, wk_s.ap().rearrange("(o p) c -> p o c", p=128))
            wv_sb = consts.tile([128, HC, D], F16)
            nc.sync.dma_start(
                wv_sb, wv_s.ap().rearrange("(o p) c -> p o c", p=128))
            for wc in range(1, 4):
                nc.scalar.dma_start(
                    wq_sb[:, 8 * wc:8 * (wc + 1), :],
                    wq_s.ap()[1024 * wc:1024 * (wc + 1), :].rearrange(
                        "(o p) c -> p o c", p=128))
            cos_sb = consts.tile([D, S], F16)
            nc.scalar.dma_start(cos_sb, cosT.ap())
            sin_sb = consts.tile([D, S], F16)
            nc.scalar.dma_start(sin_sb, sinT.ap())
            wqcos = consts.tile([D, 1], F32)
            nc.sync.dma_start(wqcos, wqcos_d.ap())
            wqsin = consts.tile([D, 1], F32)
            nc.sync.dma_start(wqsin, wqsin_d.ap())
            wkcos = consts.tile([D, 1], F32)
            nc.sync.dma_start(wkcos, wkcos_d.ap())
            wksin = consts.tile([D, 1], F32)
            nc.sync.dma_start(wksin, wksin_d.ap())
            mc = consts.tile([128, 128], F16)
            nc.scalar.dma_start(mc, tri_c.ap())
            mw = consts.tile([128, 128], F16)
            nc.scalar.dma_start(mw, tri_w.ap())
            wo_sb = consts.tile([128, QW // 128, H], F16)
            nc.scalar.dma_start(
                wo_sb, wo_s.ap().rearrange("(o p) c -> p o c", p=128))
            ones_k = consts.tile([128, 128], F16)
            nc.vector.memset(ones_k, 1.0)
            expb = consts.tile([128, 1], F32)
            nc.vector.memset(expb, EXPB)
            bias_q = consts.tile([128, 1], F32)
            nc.vector.memset(bias_q, float(D) * EPS)
            bias_k = consts.tile([128, 1], F32)
            nc.vector.memset(bias_k, EPS)

            # attention outputs, SBUF-resident until the out-projection
            scr = consts.tile([128, B, HPC, S], F16)
            # per-batch q/k/v (reused across b)
            qf = consts.tile([128, HPC, S], F16)
            k_full = consts.tile([128, S], F16)
            v_full = consts.tile([128, S // 128, 128], F16)

            # per-group norm blocks: (psum slot, kind, head idx)
            grp_norm = [
                [(0, "q", 0), (1, "q", 1), (2, "q", 2)],
                [(0, "q", 3), (1, "k", 0)],          # slot 2 of grp1 is V
            ]

            for b in range(B):
                # ---------------- QKV + norm + rope ----------------------
                # Two 3-block groups per chunk (q0q1q2 | q3,k,v), psum
                # bufs=2 so a group's epilogue overlaps the next group's
                # matmul stream. The epilogue's PE ops (ssq matmuls) are
                # deferred into the next group's stream so PE never
                # stalls on the ACT square pass.
                with tc.tile_pool(name="hs", bufs=8) as hsp, \
                     tc.tile_pool(name="epi", bufs=2) as epi, \
                     tc.tile_pool(name="ps_qkv", bufs=2, space="PSUM") as pq, \
                     tc.tile_pool(name="ps_aux", bufs=2, space="PSUM") as pa:

                    def epilogue_rest(sc, grp, qkv_ps, sqs):
                        """ssq matmuls + rsqrt + normalize + rope for one
                        group (everything after the ACT squares)."""
                        s0 = CH * sc
                        ssqs, wtils = {}, {}
                        for i, (blk, kind, hd) in enumerate(grp_norm[grp]):
                            ssq = pa.tile([128, CH], F32, tag="aux")
                            nc.tensor.matmul(ssq, ones_k, sqs[i],
                                             start=True, stop=True)
                            ssqs[i] = ssq
                        for i, (blk, kind, hd) in enumerate(grp_norm[grp]):
                            # q: rsqrt(ssq + D*eps) = rsqrt(mean+eps)/sqrt(D)
                            # (folds the 1/sqrt(D) score scale into q)
                            wtil = epi.tile([128, CH], F16, tag=f"wtil{i}")
                            if kind == "q":
                                nc.scalar.activation(wtil, ssqs[i], RSQRT,
                                                     bias=bias_q, scale=1.0)
                            else:
                                nc.scalar.activation(wtil, ssqs[i], RSQRT,
                                                     bias=bias_k,
                                                     scale=1.0 / D)
                            wtils[i] = wtil
                        for i, (blk, kind, hd) in enumerate(grp_norm[grp]):
                            qhat = epi.tile([128, CH], F16, tag=f"qhat{i}")
                            nc.vector.tensor_mul(qhat, qkv_ps[:, blk, :],
                                                 wtils[i])
                            wcos = wqcos if kind == "q" else wkcos
                            wsin = wqsin if kind == "q" else wksin
                            dst = (qf[:, hd, s0:s0 + CH] if kind == "q"
                                   else k_full[:, s0:s0 + CH])
                            t1 = epi.tile([128, CH], F16, tag=f"t1_{i}")
                            nc.vector.scalar_tensor_tensor(
                                t1, qhat, wcos, cos_sb[:, s0:s0 + CH],
                                op0=MULT, op1=MULT)
                            # in0/in1 must share a base partition (compiler
                            # constraint); sin[d] == sin[d^64] since the
                            # rope table is [freqs, freqs], so slicing sin
                            # at in0's base yields the right values.
                            t2 = epi.tile([128, CH], F16, tag=f"t2_{i}")
                            nc.vector.scalar_tensor_tensor(
                                t2[0:64, :], qhat[64:128, :], wsin[64:128, :],
                                sin_sb[64:128, s0:s0 + CH],
                                op0=MULT, op1=MULT)
                            nc.vector.scalar_tensor_tensor(
                                t2[64:128, :], qhat[0:64, :], wsin[0:64, :],
                                sin_sb[0:64, s0:s0 + CH],
                                op0=MULT, op1=MULT)
                            nc.vector.tensor_add(dst, t1, t2)

                    pending = None
                    for sc in range(NSC):
                        s0 = CH * sc
                        for grp in range(2):
                            qkv_ps = pq.tile([128, 3, CH], F32, tag="qkv")
                            for hp in range(HC // 2):
                                if hp == 3 and pending is not None:
                                    epilogue_rest(*pending)
                                    pending = None
                                ht = hsp.tile([128, 2, CH], F16, tag="ht")
                                nc.sync.dma_start(
                                    ht,
                                    hsT.ap()[b, 256 * hp:256 * (hp + 1),
                                             s0:s0 + CH].rearrange(
                                        "(o p) c -> p o c", p=128))
                                for sub in range(2):
                                    hc = 2 * hp + sub
                                    for bi in range(3):
                                        blk = 3 * grp + bi
                                        if blk < HPC:
                                            lhs = wq_sb[:, hc,
                                                        128 * blk:
                                                        128 * (blk + 1)]
                                        elif blk == HPC:
                                            lhs = wk_sb[:, hc, :]
                                        else:
                                            lhs = wv_sb[:, hc, :]
                                        nc.tensor.matmul(
                                            qkv_ps[:, bi, :], lhs,
                                            ht[:, sub, :],
                                            start=(hc == 0),
                                            stop=(hc == HC - 1))

                            if grp == 1:
                                # V: cast + DMA xbar transpose to [tok, d]
                                v16 = epi.tile([128, CH], F16, tag="v16")
                                nc.scalar.copy(v16, qkv_ps[:, 2, :])
                                for j in range(CH // 128):
                                    nc.sync.dma_start_transpose(
                                        v_full[:, 4 * sc + j, :],
                                        v16[:, 128 * j:128 * (j + 1)])
                            # ACT squares inline; the rest is deferred into
                            # the next group's matmul stream.
                            sqs = {}
                            for i, (blk, kind, hd) in enumerate(grp_norm[grp]):
                                sq = epi.tile([128, CH], F16, tag=f"sq{i}")
                                nc.scalar.activation(sq, qkv_ps[:, blk, :],
                                                     SQUARE)
                                sqs[i] = sq
                            pending = (sc, grp, qkv_ps, sqs)
                    # last group's epilogue before the pool closes
                    epilogue_rest(*pending)
                    pending = None

                # ---------------- attention for batch b ------------------
                # Emission order is engine-order: on DVE the mask adds are
                # kept ahead of the pracc accumulates (else exp_{i+1} chains
                # behind acc_i and the loop serializes), and each head's
                # tail chain (sum/clamp/recip/normalize) is deferred into
                # the next head's early stream.
                with tc.tile_pool(name="pr", bufs=6) as prp, \
                     tc.tile_pool(name="acc", bufs=2) as accp, \
                     tc.tile_pool(name="tail", bufs=2) as tlp, \
                     tc.tile_pool(name="ps_s", bufs=4, space="PSUM") as pss, \
                     tc.tile_pool(name="ps_o", bufs=2, space="PSUM") as pso, \
                     tc.tile_pool(name="ps_m", bufs=2, space="PSUM") as psm:
                    LOOK = 4
                    pend_tail = None

                    def emit_tail(b_, h_, s0_, o_ps_, pracc_):
                        # sum of probs broadcast to all partitions via the
                        # all-ones lhsT, then clamp, fast reciprocal,
                        # normalize into SBUF-resident scr.
                        sum_ps = psm.tile([128, CH], F32, tag="sum")
                        nc.tensor.matmul(sum_ps, ones_k, pracc_,
                                         start=True, stop=True)
                        # no zero-clamp: sum==0 needs all ~1024 scores below
                        # -7.3 sigma, impossible for this distribution
                        rq = tlp.tile([128, CH], F32, tag="rq")
                        nc.vector.reciprocal_approx_fast(rq, sum_ps)
                        nc.vector.tensor_mul(
                            scr[:, b_, h_, s0_:s0_ + CH], o_ps_, rq)

                    for sc in range(NSC):
                        s0 = CH * sc
                        kis = list(range(max(0, 4 * sc - 8), 4 * sc + 4))
                        n = len(kis)
                        for h in range(HPC):
                            o_ps = pso.tile([128, CH], F32, tag="o")
                            pracc = accp.tile([128, CH], F16, tag="acc")
                            prs = {}

                            def score(i):
                                ki = kis[i]
                                delta = CH * sc - 128 * ki
                                lo, hi = 0, CH
                                jm = None
                                if delta <= 0:
                                    jm, msk = -delta // 128, mc
                                    lo = 128 * jm
                                elif delta >= 640:
                                    jm, msk = (1024 - delta) // 128, mw
                                    hi = 128 * (jm + 1)
                                # compute only the unmasked q columns; the
                                # bias of -10 bounds exp even on masked
                                # entries, so masking is a cheap fp16
                                # 0/1-triangle multiply after the exp.
                                s_ps = pss.tile([128, CH], F32, tag="s")
                                nc.tensor.matmul(
                                    s_ps[:, lo:hi],
                                    k_full[:, 128 * ki:128 * (ki + 1)],
                                    qf[:, h, s0 + lo:s0 + hi],
                                    start=True, stop=True)
                                pr = prp.tile([128, CH], F16, tag="pr")
                                nc.scalar.activation(pr[:, lo:hi],
                                                     s_ps[:, lo:hi], EXP,
                                                     bias=expb)
                                if jm is not None:
                                    sub = pr[:, 128 * jm:128 * (jm + 1)]
                                    nc.vector.tensor_mul(sub, sub, msk)
                                if lo > 0:
                                    nc.gpsimd.memset(pr[:, :lo], 0.0)
                                if hi < CH:
                                    nc.gpsimd.memset(pr[:, hi:], 0.0)
                                prs[i] = pr

                            def acc(i):
                                if i == 0:
                                    nc.vector.tensor_copy(pracc, prs[i])
                                else:
                                    nc.vector.tensor_add(pracc, pracc, prs[i])

                            def pv(i):
                                nc.tensor.matmul(
                                    o_ps, v_full[:, kis[i], :], prs[i],
                                    start=(i == 0), stop=(i == n - 1))

                            for i in range(min(LOOK, n)):
                                score(i)
                            if pend_tail is not None:
                                emit_tail(*pend_tail)
                                pend_tail = None
                            for i in range(n):
                                if i + LOOK < n:
                                    score(i + LOOK)
                                acc(i)
                                pv(i)
                            pend_tail = (b, h, s0, o_ps, pracc)
                    emit_tail(*pend_tail)
                    pend_tail = None

            # ---------------- output projection ----------------------
            with tc.tile_pool(name="ostg", bufs=4) as ost, \
                 tc.tile_pool(name="ps_c", bufs=1, space="PSUM") as pc:
                NR = QW // 128
                for b in range(B):
                    for st in range(S // 128):
                        for g in range(2):
                            hcbs = range(4 * g, 4 * g + 4)
                            c_tiles = {hcb: pc.tile([128, 512], F32,
                                                    tag=f"c{hcb}",
                                                    name=f"c{hcb}")
                                       for hcb in hcbs}
                            for r in range(NR):
                                a_t = scr[:, b, r, 128 * st:128 * (st + 1)]
                                for hcb in hcbs:
                                    nc.tensor.matmul(
                                        c_tiles[hcb], a_t,
                                        wo_sb[:, r,
                                              512 * hcb:512 * (hcb + 1)],
                                        start=(r == 0), stop=(r == NR - 1))
                            for hcb in hcbs:
                                o_sb = ost.tile([128, 512], F16, tag="ostg")
                                nc.scalar.copy(o_sb, c_tiles[hcb])
                                nc.sync.dma_start(
                                    out_part.ap()[b, 128 * st:128 * (st + 1),
                                                  512 * hcb:512 * (hcb + 1)],
                                    o_sb)

    nc.compile()
    return nc


def _host_prep(hidden_states, wq, wk, wv, wo, q_norm_w, k_norm_w):
    """Build the per-core input maps (fp16 wire dtypes)."""
    f16 = np.float16
    f32 = np.float32
    hsT = np.ascontiguousarray(
        np.transpose(hidden_states, (0, 2, 1))).astype(f16)

    pos = np.arange(S, dtype=np.float64)
    inv_freq = 1.0 / (THETA ** (np.arange(0, D // 2, dtype=np.float64)
                                / (D // 2)))
    freqs = pos[None, :] * inv_freq[:, None]            # [D/2, S]
    emb = np.concatenate([freqs, freqs], axis=0)        # [D, S]
    cosT = np.cos(emb).astype(f16)
    sinT = np.sin(emb).astype(f16)

    # norm weights folded into per-partition rope scalars; the sign of
    # rotate_half folded into wsin (negative for output partitions >= 64,
    # which read input partitions < 64... sign indexed by input partition:
    # wsin[p] multiplies qhat[p] feeding output partition (p+64)%128.
    qw = q_norm_w.astype(f32)
    kw = k_norm_w.astype(f32)
    sgn = np.where(np.arange(D) < 64, 1.0, -1.0).astype(f32)
    wqcos = qw.reshape(D, 1)
    wqsin = (qw * sgn).reshape(D, 1)
    wkcos = kw.reshape(D, 1)
    wksin = (kw * sgn).reshape(D, 1)

    kd = np.arange(128)[:, None]
    qd = np.arange(128)[None, :]
    tri_c = (qd >= kd).astype(f16)
    tri_w = (qd < kd).astype(f16)

    common = {
        "hsT": hsT,
        "cosT": cosT,
        "sinT": sinT,
        "wqcos": wqcos,
        "wqsin": wqsin,
        "wkcos": wkcos,
        "wksin": wksin,
        "tri_c": tri_c,
        "tri_w": tri_w,
    }
    in_maps = []
    for c in range(NCORES):
        m = dict(common)
        m["wq_s"] = np.ascontiguousarray(wq[:, QW * c:QW * (c + 1)]).astype(f16)
        m["wk_s"] = np.ascontiguousarray(wk[:, D * c:D * (c + 1)]).astype(f16)
        m["wv_s"] = np.ascontiguousarray(wv[:, D * c:D * (c + 1)]).astype(f16)
        m["wo_s"] = np.ascontiguousarray(wo[QW * c:QW * (c + 1), :]).astype(f16)
        in_maps.append(m)
    return in_maps


def kernel(hidden_states, wq, wk, wv, wo, q_norm_w, k_norm_w,
           _trace=False, _return_results=False):
    from concourse import bass_utils

    hidden_states = np.asarray(hidden_states)
    wq, wk, wv, wo = (np.asarray(a) for a in (wq, wk, wv, wo))
    q_norm_w, k_norm_w = np.asarray(q_norm_w), np.asarray(k_norm_w)

    if "nc" not in _CACHE:
        _CACHE["nc"] = _build()
    nc = _CACHE["nc"]

    in_maps = _host_prep(hidden_states, wq, wk, wv, wo, q_norm_w, k_norm_w)
    res = bass_utils.run_bass_kernel_spmd(
        nc, in_maps, core_ids=list(range(NCORES)), trace=_trace)

    out = np.zeros((B, S, H), np.float32)
    for c in range(NCORES):
        out += res.results[c]["out_part"].astype(np.float32)
    if _return_results:
        return out, res
    return out


# revision 29
# speedup vs baseline: 1.7591x; 1.0114x over previous
"""Exaone4 attention kernel for 8 Trainium2 NeuronCores.

Sharding: tensor-parallel over heads (TP=8). Core i owns query heads
4i..4i+3 and kv head i (one GQA group), processes both batch elements,
and computes a row-parallel partial of the output projection; the host
sums the 8 partials.

Pipeline (fp16 wire dtypes, fp32 PSUM/stats):
  per batch b:
    QKV: single pass over the contraction (6 psum banks: q0..q3,k,v),
         epilogue does RMSNorm via ones-matmul + broadcast-rsqrt and
         RoPE via partition-offset DVE ops (sign folded into the sin
         table, norm weights into per-partition scalars). V transposed
         to [tok, d] tiles with DMA xbar transposes.
    attention: per (chunk, head): scores/PV matmuls depth-4 pipelined;
         exp on ACT (bias -10, sliced to the unmasked region); softmax
         denominator accumulated on DVE and reduced with one
         ones-matmul; 1/sum via DVE fast reciprocal. Outputs stay in
         SBUF (no DRAM scratch).
  out-projection: reads attention outputs straight from SBUF, writes
         fp16 partials; host sums in fp32.

Shapes (hardcoded): B=2, S=2048, H=4096, NH=32, NKV=8, D=128,
WINDOW=1024, eps=1e-5, theta=10000.
"""

import os
import sys

for _p in ("/opt/trn_rl_repo",):
    if _p not in sys.path and os.path.isdir(_p):
        sys.path.insert(0, _p)

import numpy as np

B, S, H = 2, 2048, 4096
NH, NKV, D = 32, 8, 128
WINDOW = 1024
EPS = 1e-5
THETA = 10000.0

NCORES = 8
HPC = NH // NCORES          # query heads per core = 4
QW = HPC * D                # q-proj cols per core = 512
CH = 512                    # sequence chunk
NSC = S // CH               # 4 chunks
HC = H // 128               # 32 contraction chunks
NEG = -1.0e30
EXPB = -10.0                # exp bias so fp16 probs never overflow

_CACHE = {}


def _build():
    import concourse.bass as bass
    import concourse.tile as tile
    from concourse import mybir, bacc

    F32 = mybir.dt.float32
    F16 = mybir.dt.float16
    EXP = mybir.ActivationFunctionType.Exp
    SQUARE = mybir.ActivationFunctionType.Square
    RSQRT = mybir.ActivationFunctionType.Abs_reciprocal_sqrt
    MULT = mybir.AluOpType.mult

    nc = bacc.Bacc("TRN2", target_bir_lowering=False, debug=False)

    hsT = nc.dram_tensor("hsT", [B, H, S], F16, kind="ExternalInput")
    wq_s = nc.dram_tensor("wq_s", [H, QW], F16, kind="ExternalInput")
    wk_s = nc.dram_tensor("wk_s", [H, D], F16, kind="ExternalInput")
    wv_s = nc.dram_tensor("wv_s", [H, D], F16, kind="ExternalInput")
    wo_s = nc.dram_tensor("wo_s", [QW, H], F16, kind="ExternalInput")
    cosT = nc.dram_tensor("cosT", [D, S], F16, kind="ExternalInput")
    sinT = nc.dram_tensor("sinT", [D, S], F16, kind="ExternalInput")
    wqcos_d = nc.dram_tensor("wqcos", [D, 1], F32, kind="ExternalInput")
    wqsin_d = nc.dram_tensor("wqsin", [D, 1], F32, kind="ExternalInput")
    wkcos_d = nc.dram_tensor("wkcos", [D, 1], F32, kind="ExternalInput")
    wksin_d = nc.dram_tensor("wksin", [D, 1], F32, kind="ExternalInput")
    tri_c = nc.dram_tensor("tri_c", [128, 128], F16, kind="ExternalInput")
    tri_w = nc.dram_tensor("tri_w", [128, 128], F16, kind="ExternalInput")
    out_part = nc.dram_tensor("out_part", [B, S, H], F16, kind="ExternalOutput")

    with tile.TileContext(nc) as tc, \
         nc.allow_low_precision(reason="deliberate fp16 matmul pipeline"):
        with tc.tile_pool(name="consts", bufs=1) as consts:
            # Preamble DMA order is tuned for warmup: the sync queue holds
            # only what the first matmuls need (wq chunk 0, wk, wv) before
            # the hs tiles; everything else rides the ACT queue in
            # deadline order (wq chunks 1-3, rope tables, tris, wo).
            # ~128KB per dma: consecutive dmas from one engine land on
            # different DMA rings, so fine splits transfer in parallel
            # (a single dma is capped at ~20GB/s on one ring).
            wq_sb = consts.tile([128, HC, QW], F16)
            for o in range(8):
                nc.sync.dma_start(
                    wq_sb[:, o, :], wq_s.ap()[128 * o:128 * (o + 1), :])
            wk_sb = consts.tile([128, HC, D], F16)
            wv_sb = consts.tile([128, HC, D], F16)
            for o in range(4):
                nc.sync.dma_start(
                    wk_sb[:, 8 * o:8 * (o + 1), :],
                    wk_s.ap()[1024 * o:1024 * (o + 1), :].rearrange(
                        "(o p) c -> p o c", p=128))
                nc.sync.dma_start(
                    wv_sb[:, 8 * o:8 * (o + 1), :],
                    wv_s.ap()[1024 * o:1024 * (o + 1), :].rearrange(
                        "(o p) c -> p o c", p=128))
            for wc in range(8, HC, 4):
                nc.scalar.dma_start(
                    wq_sb[:, wc:wc + 4, :],
                    wq_s.ap()[128 * wc:128 * (wc + 4), :].rearrange(
                        "(o p) c -> p o c", p=128))
            cos_sb = consts.tile([D, S], F16)
            sin_sb = consts.tile([D, S], F16)
            for o in range(2):
                nc.scalar.dma_start(cos_sb[:, 1024 * o:1024 * (o + 1)],
                                    cosT.ap()[:, 1024 * o:1024 * (o + 1)])
                nc.scalar.dma_start(sin_sb[:, 1024 * o:1024 * (o + 1)],
                                    sinT.ap()[:, 1024 * o:1024 * (o + 1)])
            wqcos = consts.tile([D, 1], F32)
            nc.sync.dma_start(wqcos, wqcos_d.ap())
            wqsin = consts.tile([D, 1], F32)
            nc.sync.dma_start(wqsin, wqsin_d.ap())
            wkcos = consts.tile([D, 1], F32)
            nc.sync.dma_start(wkcos, wkcos_d.ap())
            wksin = consts.tile([D, 1], F32)
            nc.sync.dma_start(wksin, wksin_d.ap())
            mc = consts.tile([128, 128], F16)
            nc.scalar.dma_start(mc, tri_c.ap())
            mw = consts.tile([128, 128], F16)
            nc.scalar.dma_start(mw, tri_w.ap())
            wo_sb = consts.tile([128, QW // 128, H], F16)
            for o in range(4):
                nc.scalar.dma_start(
                    wo_sb[:, o, :], wo_s.ap()[128 * o:128 * (o + 1), :])
            ones_k = consts.tile([128, 128], F16)
            nc.vector.memset(ones_k, 1.0)
            expb = consts.tile([128, 1], F32)
            nc.vector.memset(expb, EXPB)
            bias_q = consts.tile([128, 1], F32)
            nc.vector.memset(bias_q, float(D) * EPS)
            bias_k = consts.tile([128, 1], F32)
            nc.vector.memset(bias_k, EPS)

            # attention outputs, SBUF-resident until the out-projection
            scr = consts.tile([128, B, HPC, S], F16)
            # per-batch q/k/v (reused across b)
            qf = consts.tile([128, HPC, S], F16)
            k_full = consts.tile([128, S], F16)
            v_full = consts.tile([128, S // 128, 128], F16)

            # per-group norm blocks: (psum slot, kind, head idx)
            grp_norm = [
                [(0, "q", 0), (1, "q", 1), (2, "q", 2)],
                [(0, "q", 3), (1, "k", 0)],          # slot 2 of grp1 is V
            ]

            for b in range(B):
                # ---------------- QKV + norm + rope ----------------------
                # Two 3-block groups per chunk (q0q1q2 | q3,k,v), psum
                # bufs=2 so a group's epilogue overlaps the next group's
                # matmul stream. The epilogue's PE ops (ssq matmuls) are
                # deferred into the next group's stream so PE never
                # stalls on the ACT square pass.
                with tc.tile_pool(name="hs", bufs=8) as hsp, \
                     tc.tile_pool(name="epi", bufs=2) as epi, \
                     tc.tile_pool(name="ps_qkv", bufs=2, space="PSUM") as pq, \
                     tc.tile_pool(name="ps_aux", bufs=2, space="PSUM") as pa:

                    def epilogue_rest(sc, grp, qkv_ps, sqs):
                        """ssq matmuls + rsqrt + normalize + rope for one
                        group (everything after the ACT squares)."""
                        s0 = CH * sc
                        ssqs, wtils = {}, {}
                        for i, (blk, kind, hd) in enumerate(grp_norm[grp]):
                            ssq = pa.tile([128, CH], F32, tag="aux")
                            nc.tensor.matmul(ssq, ones_k, sqs[i],
                                             start=True, stop=True)
                            ssqs[i] = ssq
                        for i, (blk, kind, hd) in enumerate(grp_norm[grp]):
                            # q: rsqrt(ssq + D*eps) = rsqrt(mean+eps)/sqrt(D)
                            # (folds the 1/sqrt(D) score scale into q)
                            wtil = epi.tile([128, CH], F16, tag=f"wtil{i}")
                            if kind == "q":
                                nc.scalar.activation(wtil, ssqs[i], RSQRT,
                                                     bias=bias_q, scale=1.0)
                            else:
                                nc.scalar.activation(wtil, ssqs[i], RSQRT,
                                                     bias=bias_k,
                                                     scale=1.0 / D)
                            wtils[i] = wtil
                        for i, (blk, kind, hd) in enumerate(grp_norm[grp]):
                            qhat = epi.tile([128, CH], F16, tag=f"qhat{i}")
                            nc.vector.tensor_mul(qhat, qkv_ps[:, blk, :],
                                                 wtils[i])
                            wcos = wqcos if kind == "q" else wkcos
                            wsin = wqsin if kind == "q" else wksin
                            dst = (qf[:, hd, s0:s0 + CH] if kind == "q"
                                   else k_full[:, s0:s0 + CH])
                            t1 = epi.tile([128, CH], F16, tag=f"t1_{i}")
                            nc.vector.scalar_tensor_tensor(
                                t1, qhat, wcos, cos_sb[:, s0:s0 + CH],
                                op0=MULT, op1=MULT)
                            # in0/in1 must share a base partition (compiler
                            # constraint); sin[d] == sin[d^64] since the
                            # rope table is [freqs, freqs], so slicing sin
                            # at in0's base yields the right values.
                            t2 = epi.tile([128, CH], F16, tag=f"t2_{i}")
                            nc.vector.scalar_tensor_tensor(
                                t2[0:64, :], qhat[64:128, :], wsin[64:128, :],
                                sin_sb[64:128, s0:s0 + CH],
                                op0=MULT, op1=MULT)
                            nc.vector.scalar_tensor_tensor(
                                t2[64:128, :], qhat[0:64, :], wsin[0:64, :],
                                sin_sb[0:64, s0:s0 + CH],
                                op0=MULT, op1=MULT)
                            nc.vector.tensor_add(dst, t1, t2)

                    pending = None
                    for sc in range(NSC):
                        s0 = CH * sc
                        for grp in range(2):
                            qkv_ps = pq.tile([128, 3, CH], F32, tag="qkv")
                            for hp in range(HC // 2):
                                if hp == 3 and pending is not None:
                                    epilogue_rest(*pending)
                                    pending = None
                                ht = hsp.tile([128, 2, CH], F16, tag="ht")
                                nc.sync.dma_start(
                                    ht,
                                    hsT.ap()[b, 256 * hp:256 * (hp + 1),
                                             s0:s0 + CH].rearrange(
                                        "(o p) c -> p o c", p=128))
                                for sub in range(2):
                                    hc = 2 * hp + sub
                                    for bi in range(3):
                                        blk = 3 * grp + bi
                                        if blk < HPC:
                                            lhs = wq_sb[:, hc,
                                                        128 * blk:
                                                        128 * (blk + 1)]
                                        elif blk == HPC:
                                            lhs = wk_sb[:, hc, :]
                                        else:
                                            lhs = wv_sb[:, hc, :]
                                        nc.tensor.matmul(
                                            qkv_ps[:, bi, :], lhs,
                                            ht[:, sub, :],
                                            start=(hc == 0),
                                            stop=(hc == HC - 1))

                            if grp == 1:
                                # V: cast + DMA xbar transpose to [tok, d]
                                v16 = epi.tile([128, CH], F16, tag="v16")
                                nc.scalar.copy(v16, qkv_ps[:, 2, :])
                                for j in range(CH // 128):
                                    nc.sync.dma_start_transpose(
                                        v_full[:, 4 * sc + j, :],
                                        v16[:, 128 * j:128 * (j + 1)])
                            # ACT squares inline; the rest is deferred into
                            # the next group's matmul stream.
                            sqs = {}
                            for i, (blk, kind, hd) in enumerate(grp_norm[grp]):
                                sq = epi.tile([128, CH], F16, tag=f"sq{i}")
                                nc.scalar.activation(sq, qkv_ps[:, blk, :],
                                                     SQUARE)
                                sqs[i] = sq
                            pending = (sc, grp, qkv_ps, sqs)
                    # last group's epilogue before the pool closes
                    epilogue_rest(*pending)
                    pending = None

                # ---------------- attention for batch b ------------------
                # Emission order is engine-order: on DVE the mask adds are
                # kept ahead of the pracc accumulates (else exp_{i+1} chains
                # behind acc_i and the loop serializes), and each head's
                # tail chain (sum/clamp/recip/normalize) is deferred into
                # the next head's early stream.
                with tc.tile_pool(name="pr", bufs=6) as prp, \
                     tc.tile_pool(name="acc", bufs=2) as accp, \
                     tc.tile_pool(name="tail", bufs=2) as tlp, \
                     tc.tile_pool(name="ps_s", bufs=4, space="PSUM") as pss, \
                     tc.tile_pool(name="ps_o", bufs=2, space="PSUM") as pso, \
                     tc.tile_pool(name="ps_m", bufs=2, space="PSUM") as psm:
                    LOOK = 4
                    pend_tail = None

                    def emit_tail(b_, h_, s0_, o_ps_, pracc_):
                        # sum of probs broadcast to all partitions via the
                        # all-ones lhsT, then clamp, fast reciprocal,
                        # normalize into SBUF-resident scr.
                        sum_ps = psm.tile([128, CH], F32, tag="sum")
                        nc.tensor.matmul(sum_ps, ones_k, pracc_,
                                         start=True, stop=True)
                        # no zero-clamp: sum==0 needs all ~1024 scores below
                        # -7.3 sigma, impossible for this distribution
                        rq = tlp.tile([128, CH], F32, tag="rq")
                        nc.vector.reciprocal_approx_fast(rq, sum_ps)
                        nc.vector.tensor_mul(
                            scr[:, b_, h_, s0_:s0_ + CH], o_ps_, rq)

                    for sc in range(NSC):
                        s0 = CH * sc
                        kis = list(range(max(0, 4 * sc - 8), 4 * sc + 4))
                        n = len(kis)
                        for h in range(HPC):
                            o_ps = pso.tile([128, CH], F32, tag="o")
                            pracc = accp.tile([128, CH], F16, tag="acc")
                            prs = {}
                            bounds = {}

                            def score(i):
                                ki = kis[i]
                                delta = CH * sc - 128 * ki
                                lo, hi = 0, CH
                                jm = None
                                if delta <= 0:
                                    jm, msk = -delta // 128, mc
                                    lo = 128 * jm
                                elif delta >= 640:
                                    jm, msk = (1024 - delta) // 128, mw
                                    hi = 128 * (jm + 1)
                                # compute only the unmasked q columns; the
                                # bias of -10 bounds exp even on masked
                                # entries, so masking is a cheap fp16
                                # 0/1-triangle multiply after the exp.
                                s_ps = pss.tile([128, CH], F32, tag="s")
                                nc.tensor.matmul(
                                    s_ps[:, lo:hi],
                                    k_full[:, 128 * ki:128 * (ki + 1)],
                                    qf[:, h, s0 + lo:s0 + hi],
                                    start=True, stop=True)
                                pr = prp.tile([128, CH], F16, tag="pr")
                                nc.scalar.activation(pr[:, lo:hi],
                                                     s_ps[:, lo:hi], EXP,
                                                     bias=expb)
                                if jm is not None:
                                    sub = pr[:, 128 * jm:128 * (jm + 1)]
                                    nc.vector.tensor_mul(sub, sub, msk)
                                if lo > 0:
                                    nc.gpsimd.memset(pr[:, :lo], 0.0)
                                if hi < CH:
                                    nc.gpsimd.memset(pr[:, hi:], 0.0)
                                prs[i] = pr
                                bounds[i] = (lo, hi)

                            def acc(i):
                                # accumulate only the valid columns (the
                                # rest of pr is zero anyway)
                                lo, hi = bounds[i]
                                if i == 0:
                                    if lo > 0:
                                        nc.gpsimd.memset(pracc[:, :lo], 0.0)
                                    if hi < CH:
                                        nc.gpsimd.memset(pracc[:, hi:], 0.0)
                                    nc.vector.tensor_copy(
                                        pracc[:, lo:hi], prs[i][:, lo:hi])
                                else:
                                    nc.vector.tensor_add(
                                        pracc[:, lo:hi], pracc[:, lo:hi],
                                        prs[i][:, lo:hi])

                            def pv(i):
                                nc.tensor.matmul(
                                    o_ps, v_full[:, kis[i], :], prs[i],
                                    start=(i == 0), stop=(i == n - 1))

                            for i in range(min(LOOK, n)):
                                score(i)
                            if pend_tail is not None:
                                emit_tail(*pend_tail)
                                pend_tail = None
                            for i in range(n):
                                if i + LOOK < n:
                                    score(i + LOOK)
                                acc(i)
                                pv(i)
                            pend_tail = (b, h, s0, o_ps, pracc)
                    emit_tail(*pend_tail)
                    pend_tail = None

            # ---------------- output projection ----------------------
            with tc.tile_pool(name="ostg", bufs=4) as ost, \
                 tc.tile_pool(name="ps_c", bufs=1, space="PSUM") as pc:
                NR = QW // 128
                for b in range(B):
                    for st in range(S // 128):
                        for g in range(2):
                            hcbs = range(4 * g, 4 * g + 4)
                            c_tiles = {hcb: pc.tile([128, 512], F32,
                                                    tag=f"c{hcb}",
                                                    name=f"c{hcb}")
                                       for hcb in hcbs}
                            for r in range(NR):
                                a_t = scr[:, b, r, 128 * st:128 * (st + 1)]
                                for hcb in hcbs:
                                    nc.tensor.matmul(
                                        c_tiles[hcb], a_t,
                                        wo_sb[:, r,
                                              512 * hcb:512 * (hcb + 1)],
                                        start=(r == 0), stop=(r == NR - 1))
                            for hcb in hcbs:
                                o_sb = ost.tile([128, 512], F16, tag="ostg")
                                nc.scalar.copy(o_sb, c_tiles[hcb])
                                nc.sync.dma_start(
                                    out_part.ap()[b, 128 * st:128 * (st + 1),
                                                  512 * hcb:512 * (hcb + 1)],
                                    o_sb)

    nc.compile()
    return nc


def _host_prep(hidden_states, wq, wk, wv, wo, q_norm_w, k_norm_w):
    """Build the per-core input maps (fp16 wire dtypes)."""
    f16 = np.float16
    f32 = np.float32
    hsT = np.ascontiguousarray(
        np.transpose(hidden_states, (0, 2, 1))).astype(f16)

    pos = np.arange(S, dtype=np.float64)
    inv_freq = 1.0 / (THETA ** (np.arange(0, D // 2, dtype=np.float64)
                                / (D // 2)))
    freqs = pos[None, :] * inv_freq[:, None]            # [D/2, S]
    emb = np.concatenate([freqs, freqs], axis=0)        # [D, S]
    cosT = np.cos(emb).astype(f16)
    sinT = np.sin(emb).astype(f16)

    # norm weights folded into per-partition rope scalars; the sign of
    # rotate_half folded into wsin (negative for output partitions >= 64,
    # which read input partitions < 64... sign indexed by input partition:
    # wsin[p] multiplies qhat[p] feeding output partition (p+64)%128.
    qw = q_norm_w.astype(f32)
    kw = k_norm_w.astype(f32)
    sgn = np.where(np.arange(D) < 64, 1.0, -1.0).astype(f32)
    wqcos = qw.reshape(D, 1)
    wqsin = (qw * sgn).reshape(D, 1)
    wkcos = kw.reshape(D, 1)
    wksin = (kw * sgn).reshape(D, 1)

    kd = np.arange(128)[:, None]
    qd = np.arange(128)[None, :]
    tri_c = (qd >= kd).astype(f16)
    tri_w = (qd < kd).astype(f16)

    common = {
        "hsT": hsT,
        "cosT": cosT,
        "sinT": sinT,
        "wqcos": wqcos,
        "wqsin": wqsin,
        "wkcos": wkcos,
        "wksin": wksin,
        "tri_c": tri_c,
        "tri_w": tri_w,
    }
    in_maps = []
    for c in range(NCORES):
        m = dict(common)
        m["wq_s"] = np.ascontiguousarray(wq[:, QW * c:QW * (c + 1)]).astype(f16)
        m["wk_s"] = np.ascontiguousarray(wk[:, D * c:D * (c + 1)]).astype(f16)
        m["wv_s"] = np.ascontiguousarray(wv[:, D * c:D * (c + 1)]).astype(f16)
        m["wo_s"] = np.ascontiguousarray(wo[QW * c:QW * (c + 1), :]).astype(f16)
        in_maps.append(m)
    return in_maps


def kernel(hidden_states, wq, wk, wv, wo, q_norm_w, k_norm_w,
           _trace=False, _return_results=False):
    from concourse import bass_utils

    hidden_states = np.asarray(hidden_states)
    wq, wk, wv, wo = (np.asarray(a) for a in (wq, wk, wv, wo))
    q_norm_w, k_norm_w = np.asarray(q_norm_w), np.asarray(k_norm_w)

    if "nc" not in _CACHE:
        _CACHE["nc"] = _build()
    nc = _CACHE["nc"]

    in_maps = _host_prep(hidden_states, wq, wk, wv, wo, q_norm_w, k_norm_w)
    res = bass_utils.run_bass_kernel_spmd(
        nc, in_maps, core_ids=list(range(NCORES)), trace=_trace)

    out = np.zeros((B, S, H), np.float32)
    for c in range(NCORES):
        out += res.results[c]["out_part"].astype(np.float32)
    if _return_results:
        return out, res
    return out
